# revision 1
# baseline (speedup 1.0000x reference)
"""Trainium2 Bass kernel for nn_DFANet (analog PIM crossbar MLP emulation).

Sharding: input-bit-plane parallel — core c owns input bit i=c for layer 1
and hq bit i2=c for layer 2. All ADC min/max groups are then core-local;
one fp32 sum-AllReduce of the accumulator happens at each layer boundary.

Self-contained: hardcodes all shapes; host precomputes bit-planes and
conductance tensors (exact fp32 mirror of the reference formulas, split
into bf16 hi+lo pairs so PE products with 0/1 bits are fp32-grade).
"""
import math
import sys

import numpy as np

sys.path.insert(0, "/opt/trn_rl_repo")

import ml_dtypes  # noqa: E402
import concourse.bass as bass  # noqa: E402
import concourse.mybir as mybir  # noqa: E402
import concourse.tile as tile  # noqa: E402
import concourse.bacc as bacc  # noqa: E402
from concourse import bass_utils  # noqa: E402

F32 = mybir.dt.float32
BF16 = mybir.dt.bfloat16
I32 = mybir.dt.int32
AX = mybir.AxisListType
OP = mybir.AluOpType
ACTF = mybir.ActivationFunctionType

# problem constants
I_BITS = 8
S1, S2 = 7, 4
KSL = 4            # weight slices
F1, F2 = 512, 10
N1, N2 = 784, 512
NP1 = S1 * 128
CR = 4.0
LOWER, UPPER = np.float32(1.0 / 10.0), np.float32(1.0)
GLO = np.float32(np.float32(CR - 1.0) * LOWER)      # (cr-1)*lower
GSC = np.float32(UPPER - LOWER)                     # 0.9
KAPPA = float(np.float32(2.0 / (0.9 * 255.0 * 255.0)))
RSCALE = float(np.float32(32.0 * (1.0 - 2.0 ** -22)))
STEPS = float(np.float32(2.0 ** -5))
C03 = float(np.float32(3.0) * np.float32(0.1))      # (cr-1)*lower as f32

_NC_CACHE = {}


# ----------------------------------------------------------------- host prep
def _qweights(w):
    """Xi -> slices -> conductances, mirroring reference fp32 ops exactly."""
    w = np.asarray(w, np.float32)
    Xi = np.clip(np.round((w + np.float32(1.0)) * np.float32(0.5) * np.float32(255.0)),
                 0.0, 255.0).astype(np.float32)
    return Xi


def _gtensor(Xi, noise, S):
    """g[f, s*128+a, k] fp32, padded to S*128 rows; mirrors reference."""
    F, N = Xi.shape
    Np = S * 128
    Xi = np.pad(Xi, ((0, 0), (0, Np - N)))
    kpow = (np.float32(CR) ** np.arange(KSL)).astype(np.float32)
    slc = np.mod(np.floor(Xi[..., None] / kpow), np.float32(CR)).astype(np.float32)
    g = slc * GSC + GLO
    g = (g * (np.float32(1.0) + np.float32(0.05) * np.asarray(noise, np.float32))).astype(np.float32)
    return g  # [F, Np, K]


def _hi_lo(x):
    hi = x.astype(ml_dtypes.bfloat16)
    lo = (x - hi.astype(np.float32)).astype(ml_dtypes.bfloat16)
    return hi, lo


def host_prepare(x, w1, w3, noise1, noise3, B):
    """Returns (shared dict, per-core list of dicts) of DRAM input arrays."""
    x = np.asarray(x, np.float32)[:B]
    xq = np.round(np.clip(x, 0.0, 1.0) * np.float32(255.0)).astype(np.float32)  # [B, N1]
    xq_pad = np.pad(xq, ((0, 0), (0, NP1 - N1)))
    zpow = (np.float32(2.0) ** np.arange(I_BITS)).astype(np.float32)
    bits = np.mod(np.floor(xq_pad[..., None] / zpow), np.float32(2.0))  # [B, NP1, I]
    # bitsT per i: [128, S1*B] bf16, block s cols = bits[:, s*128+a, i].T
    bitsT = np.transpose(bits, (2, 1, 0))  # [I, NP1, B]
    bitsT = bitsT.reshape(I_BITS, S1, 128, B)

    g1 = _gtensor(_qweights(w1), noise1, S1)          # [512, 896, 4]
    # lhsT layout per (k,s): [a=128, f=512]; slot sk = k*7+s
    g1l = np.transpose(g1.reshape(F1, S1, 128, KSL), (3, 1, 2, 0))  # [K, S1, 128, F1]
    g1flat = g1l.reshape(KSL * S1, 128, F1).transpose(1, 0, 2).reshape(128, KSL * S1 * F1)
    g1hi, g1lo = _hi_lo(np.ascontiguousarray(g1flat))

    g2 = _gtensor(_qweights(w3), noise3, S2)          # [10, 512, 4]
    # per s2: [a=128, 40] with col k*10+f
    g2l = np.transpose(g2.reshape(F2, S2, 128, KSL), (1, 2, 3, 0))  # [S2, 128, K, F2]
    g2flat = g2l.reshape(S2, 128, KSL * F2).transpose(1, 0, 2).reshape(128, S2 * KSL * F2)
    g2hi, g2lo = _hi_lo(np.ascontiguousarray(g2flat))

    xqsum = xq.sum(axis=1, dtype=np.float32).astype(np.float32)   # [B]
    row1 = (-(xqsum / np.float32(255.0)) / np.float32(KAPPA) / np.float32(8.0)
            ).astype(np.float32)[None, :]  # [1, B]

    e7 = np.zeros((128, S1 * S1), np.float32)
    for s in range(S1):
        e7[:, s * S1 + s] = 1.0
    e42 = np.zeros((128, S2 * S2), np.float32)
    for s in range(S2):
        e42[:, s * S2 + s] = 1.0
    ones128 = np.ones((128, 1), np.float32)
    onesrow = np.ones((1, 128), np.float32)
    ones7 = np.ones((S1, 1), np.float32)
    fold40 = np.zeros((KSL * F2, F2), np.float32)
    for k in range(KSL):
        for j in range(F2):
            fold40[k * F2 + j, j] = 1.0
    blockind = np.zeros((KSL, KSL * F2), np.float32)
    for k in range(KSL):
        blockind[k, k * F2:(k + 1) * F2] = 1.0
    ident = np.eye(128, dtype=np.float32)
    scal2t = np.zeros((S2, KSL), np.float32)

    shared = dict(
        g1hi=np.asarray(g1hi), g1lo=np.asarray(g1lo),
        g2hi=np.asarray(g2hi), g2lo=np.asarray(g2lo),
        e7=e7.astype(ml_dtypes.bfloat16), e42=e42.astype(ml_dtypes.bfloat16),
        ones128=ones128.astype(ml_dtypes.bfloat16),
        onesrow=onesrow, ones7=ones7, fold40=fold40, blockind=blockind,
        ident=ident, row1=row1,
    )
    per_core = []
    for c in range(8):
        sc = np.float32(2.0 ** c)
        cconst = np.zeros((128, 16), np.float32)
        for k in range(KSL):
            cconst[:, k] = sc * np.float32(4.0 ** k)
        cconst[:, 4] = -np.float32(85.0) * sc
        cconst[:, 5] = np.float32(2.0 ** -c)
        st2 = scal2t.copy()
        for k in range(KSL):
            st2[:, k] = sc * np.float32(4.0 ** k)
        btc = np.ascontiguousarray(
            bitsT[c].transpose(1, 0, 2).reshape(128, S1 * B)).astype(ml_dtypes.bfloat16)
        per_core.append(dict(bitsT=np.asarray(btc), cconst=cconst, scal2t=st2))
    return shared, per_core


# ------------------------------------------------------------- bass program
def build_nc(B):
    if B in _NC_CACHE:
        return _NC_CACHE[B]
    BH = B // 512 if B >= 512 else 1
    NB = min(B, 512)                      # matmul moving chunk
    nc = bacc.Bacc("TRN2", target_bir_lowering=False, debug=False,
                   num_devices=8)

    def din(name, shape, dt):
        return nc.dram_tensor(name, list(shape), dt, kind="ExternalInput")[:]

    bitsT = din("bitsT", (128, S1 * B), BF16)
    g1hi = din("g1hi", (128, KSL * S1 * F1), BF16)
    g1lo = din("g1lo", (128, KSL * S1 * F1), BF16)
    g2hi = din("g2hi", (128, S2 * KSL * F2), BF16)
    g2lo = din("g2lo", (128, S2 * KSL * F2), BF16)
    e7 = din("e7", (128, S1 * S1), BF16)
    e42 = din("e42", (128, S2 * S2), BF16)
    ones128 = din("ones128", (128, 1), BF16)
    onesrow = din("onesrow", (1, 128), F32)
    ones7 = din("ones7", (S1, 1), F32)
    fold40 = din("fold40", (KSL * F2, F2), F32)
    blockind = din("blockind", (KSL, KSL * F2), F32)
    ident = din("ident", (128, 128), F32)
    row1 = din("row1", (1, B), F32)
    cconst = din("cconst", (128, 16), F32)
    scal2t = din("scal2t", (S2, KSL), F32)

    out2d = nc.dram_tensor("out2", [F2, B], F32, kind="ExternalOutput")[:]
    hdbg = nc.dram_tensor("h_dbg", [128, 4 * B], F32, kind="ExternalOutput")[:]

    with tile.TileContext(nc) as tc:
        with (
            tc.tile_pool(name="const", bufs=1) as cp,
            tc.tile_pool(name="work", bufs=4) as wp,
            tc.tile_pool(name="idx", bufs=3) as ip,
            tc.tile_pool(name="tiny", bufs=16) as tp,
            tc.tile_pool(name="coll", bufs=2) as lp,
            tc.tile_pool(name="rows", bufs=3) as rp,
            tc.tile_pool(name="ps_a", bufs=2, space="PSUM") as psA,
            tc.tile_pool(name="ps_b", bufs=1, space="PSUM") as psB,
            tc.tile_pool(name="ps_m", bufs=1, space="PSUM") as psM,
            tc.tile_pool(name="dram", bufs=1, space="DRAM") as dp,
        ):
            # ---- load constants to SBUF
            def load(ap, shape, dt, tag):
                t = cp.tile(list(shape), dt, tag=tag)
                nc.sync.dma_start(t[:], ap)
                return t

            sb_bits = load(bitsT, (128, S1 * B), BF16, tag="sb_bits")
            sb_g1h = load(g1hi, (128, KSL * S1 * F1), BF16, tag="sb_g1h")
            sb_g1l = load(g1lo, (128, KSL * S1 * F1), BF16, tag="sb_g1l")
            sb_g2h = load(g2hi, (128, S2 * KSL * F2), BF16, tag="sb_g2h")
            sb_g2l = load(g2lo, (128, S2 * KSL * F2), BF16, tag="sb_g2l")
            sb_e7 = load(e7, (128, S1 * S1), BF16, tag="sb_e7")
            sb_e42 = load(e42, (128, S2 * S2), BF16, tag="sb_e42")
            sb_o128 = load(ones128, (128, 1), BF16, tag="sb_o128")
            sb_orow = load(onesrow, (1, 128), F32, tag="sb_orow")
            sb_o7 = load(ones7, (S1, 1), F32, tag="sb_o7")
            sb_f40 = load(fold40, (KSL * F2, F2), F32, tag="sb_f40")
            sb_bind = load(blockind, (KSL, KSL * F2), F32, tag="sb_bind")
            sb_id = load(ident, (128, 128), F32, tag="sb_id")
            sb_row1 = load(row1, (1, B), F32, tag="sb_row1")
            sb_cc = load(cconst, (128, 16), F32, tag="sb_cc")
            sb_s2t = load(scal2t, (S2, KSL), F32, tag="sb_s2t")

            acc1 = cp.tile([128, 4 * B], F32)
            sigc = cp.tile([S1, KSL], F32)

            # PE warm-up on every DMA-loaded constant it will read later, so
            # later matmuls don't need a third (DMA) sync-wait slot.
            warm = psM.tile([1, 16], F32, tag="m")
            for j, t in enumerate([sb_id, sb_bind, sb_f40, sb_orow, sb_o7,
                                   sb_g1h, sb_g1l, sb_g2h, sb_g2l, sb_e7,
                                   sb_e42, sb_o128, sb_bits, sb_row1]):
                nc.tensor.matmul(warm[0:1, j:j + 1], t[0:1, 0:1], t[0:1, 0:1],
                                 start=True, stop=True)

            def mm_group1(ps, k, s):
                """4 bf16 MMs computing P for group (k,s): ps[fcpart? no—
                ps is [128,B] for one fc]  -- caller loops fc."""
                pass

            def emit_p1(ps, wslice_hi, wslice_lo, rhs_base):
                for h in range(BH):
                    nc.tensor.matmul(ps[:, h * NB:(h + 1) * NB], wslice_hi,
                                     sb_bits[:, rhs_base + h * NB: rhs_base + (h + 1) * NB],
                                     start=True, stop=False)
                for h in range(BH):
                    nc.tensor.matmul(ps[:, h * NB:(h + 1) * NB], wslice_lo,
                                     sb_bits[:, rhs_base + h * NB: rhs_base + (h + 1) * NB],
                                     start=False, stop=True)

            # ---------------- dummy D1 ----------------
            psD = psM.tile([S1, B], F32, tag="m")
            for s in range(S1):
                for h in range(BH):
                    nc.tensor.matmul(psD[:, h * NB:(h + 1) * NB],
                                     sb_e7[:, s * S1:(s + 1) * S1],
                                     sb_bits[:, s * B + h * NB: s * B + (h + 1) * NB],
                                     start=(s == 0), stop=(s == S1 - 1))
            Dsb = wp.tile([S1, B], F32, tag="w32")
            nc.vector.tensor_scalar(out=Dsb[:], in0=psD[:], scalar1=C03, scalar2=0.0,
                                    op0=OP.mult, op1=OP.add)
            mxD = tp.tile([S1, 1], F32, tag="t")
            mnD = tp.tile([S1, 1], F32, tag="t")
            nc.vector.tensor_reduce(out=mxD[:], in_=Dsb[:], axis=AX.X, op=OP.max)
            nc.vector.tensor_reduce(out=mnD[:], in_=Dsb[:], axis=AX.X, op=OP.min)
            dD = tp.tile([S1, 1], F32, tag="t")
            nc.vector.tensor_tensor(out=dD[:], in0=mxD[:], in1=mnD[:], op=OP.subtract)
            rcD = tp.tile([S1, 1], F32, tag="t")
            nc.vector.reciprocal(rcD[:], dD[:])
            mkD = tp.tile([S1, 1], F32, tag="t")
            nc.vector.tensor_scalar(out=mkD[:], in0=dD[:], scalar1=0.0, scalar2=0.0,
                                    op0=OP.is_gt, op1=OP.add)
            rD = tp.tile([S1, 1], F32, tag="t")
            nc.vector.tensor_scalar(out=rD[:], in0=rcD[:], scalar1=mkD[:, 0:1],
                                    scalar2=RSCALE, op0=OP.mult, op1=OP.mult)
            rDn = tp.tile([S1, 1], F32, tag="t")
            nc.vector.tensor_scalar(out=rDn[:], in0=rD[:], scalar1=-1.0, scalar2=0.0,
                                    op0=OP.mult, op1=OP.add)
            bD = tp.tile([S1, 1], F32, tag="t")
            nc.vector.tensor_scalar(out=bD[:], in0=mnD[:], scalar1=rDn[:, 0:1],
                                    scalar2=-0.5, op0=OP.mult, op1=OP.add)
            stD = tp.tile([S1, 1], F32, tag="t")
            nc.vector.tensor_scalar(out=stD[:], in0=dD[:], scalar1=STEPS, scalar2=0.0,
                                    op0=OP.mult, op1=OP.add)
            idxD = wp.tile([S1, B], I32, tag="wi32")
            nc.vector.tensor_scalar(out=idxD[:], in0=Dsb[:], scalar1=rD[:, 0:1],
                                    scalar2=bD[:, 0:1], op0=OP.mult, op1=OP.add)
            DqD = wp.tile([S1, B], F32, tag="w32")
            nc.vector.tensor_scalar(out=DqD[:], in0=idxD[:], scalar1=stD[:, 0:1],
                                    scalar2=mnD[:, 0:1], op0=OP.mult, op1=OP.add)

            # ---------------- layer-1 main loop ----------------
            for k in range(KSL):
                maxC = lp.tile([128, S1 * KSL], F32, tag="mx")
                minC = lp.tile([128, S1 * KSL], F32, tag="mn")
                # pass 1
                for s in range(S1):
                    for fc in range(4):
                        ps = psA.tile([128, B], F32, tag="p1")
                        wof = (k * S1 + s) * F1 + fc * 128
                        emit_p1(ps, sb_g1h[:, wof:wof + 128], sb_g1l[:, wof:wof + 128],
                                s * B)
                        nc.vector.tensor_reduce(out=maxC[:, s * 4 + fc:s * 4 + fc + 1],
                                                in_=ps[:], axis=AX.X, op=OP.max)
                        nc.vector.tensor_reduce(out=minC[:, s * 4 + fc:s * 4 + fc + 1],
                                                in_=ps[:], axis=AX.X, op=OP.min)
                # combine k: fc-fold then transpose then partition fold
                red = tp.tile([128, 2 * S1], F32, tag="red")
                nc.vector.tensor_reduce(out=red[:, 0:S1],
                                        in_=maxC[:].rearrange("p (s f) -> p s f", f=4),
                                        axis=AX.X, op=OP.max)
                nc.vector.tensor_reduce(out=red[:, S1:2 * S1],
                                        in_=minC[:].rearrange("p (s f) -> p s f", f=4),
                                        axis=AX.X, op=OP.min)
                ptm = psM.tile([S1, 128], F32, tag="m")
                nc.tensor.transpose(ptm[:], red[:, 0:S1], sb_id[:])
                tcm = tp.tile([S1, 128], F32, tag="tc")
                nc.vector.tensor_copy(tcm[:], ptm[:])
                ptn = psM.tile([S1, 128], F32, tag="m")
                nc.tensor.transpose(ptn[:], red[:, S1:2 * S1], sb_id[:])
                tcn = tp.tile([S1, 128], F32, tag="tc")
                nc.vector.tensor_copy(tcn[:], ptn[:])
                mx = tp.tile([S1, 1], F32, tag="t")
                mn = tp.tile([S1, 1], F32, tag="t")
                nc.vector.tensor_reduce(out=mx[:], in_=tcm[:], axis=AX.X, op=OP.max)
                nc.vector.tensor_reduce(out=mn[:], in_=tcn[:], axis=AX.X, op=OP.min)
                d = tp.tile([S1, 1], F32, tag="t")
                nc.vector.tensor_tensor(out=d[:], in0=mx[:], in1=mn[:], op=OP.subtract)
                rc = tp.tile([S1, 1], F32, tag="t")
                nc.vector.reciprocal(rc[:], d[:])
                mk = tp.tile([S1, 1], F32, tag="t")
                nc.vector.tensor_scalar(out=mk[:], in0=d[:], scalar1=0.0, scalar2=0.0,
                                        op0=OP.is_gt, op1=OP.add)
                rr = tp.tile([S1, 1], F32, tag="t")
                nc.vector.tensor_scalar(out=rr[:], in0=rc[:], scalar1=mk[:, 0:1],
                                        scalar2=RSCALE, op0=OP.mult, op1=OP.mult)
                rrn = tp.tile([S1, 1], F32, tag="t")
                nc.vector.tensor_scalar(out=rrn[:], in0=rr[:], scalar1=-1.0, scalar2=0.0,
                                        op0=OP.mult, op1=OP.add)
                bb = tp.tile([S1, 1], F32, tag="t")
                nc.vector.tensor_scalar(out=bb[:], in0=mn[:], scalar1=rrn[:, 0:1],
                                        scalar2=-0.5, op0=OP.mult, op1=OP.add)
                stp = tp.tile([S1, 1], F32, tag="t")
                nc.vector.tensor_scalar(out=stp[:], in0=d[:], scalar1=STEPS, scalar2=0.0,
                                        op0=OP.mult, op1=OP.add)
                cc = tp.tile([S1, 1], F32, tag="t")
                nc.vector.tensor_scalar(out=cc[:], in0=stp[:], scalar1=sb_cc[0:S1, k:k + 1],
                                        scalar2=0.0, op0=OP.mult, op1=OP.add)
                nc.vector.tensor_scalar(out=sigc[:, k:k + 1], in0=mn[:],
                                        scalar1=sb_cc[0:S1, k:k + 1], scalar2=0.0,
                                        op0=OP.mult, op1=OP.add)
                # broadcast r/b/c to [128, 3*S1]: transpose cols to one row, rank-1
                prow = psM.tile([1, 3 * S1], F32, tag="m")
                nc.tensor.transpose(prow[:, 0:S1], rr[:], sb_id[0:S1, 0:S1])
                nc.tensor.transpose(prow[:, S1:2 * S1], bb[:], sb_id[0:S1, 0:S1])
                nc.tensor.transpose(prow[:, 2 * S1:3 * S1], cc[:], sb_id[0:S1, 0:S1])
                row21 = tp.tile([1, 3 * S1], F32, tag="r21")
                nc.vector.tensor_copy(row21[:], prow[:])
                pbc = psM.tile([128, 3 * S1], F32, tag="m")
                nc.tensor.matmul(pbc[:], sb_orow[:], row21[:], start=True, stop=True)
                bck = tp.tile([128, 3 * S1], F32, tag="bck")
                nc.vector.tensor_copy(bck[:], pbc[:])
                # pass 2
                for s in range(S1):
                    for fc in range(4):
                        ps = psB.tile([128, B], F32, tag="p2")
                        wof = (k * S1 + s) * F1 + fc * 128
                        emit_p1(ps, sb_g1h[:, wof:wof + 128], sb_g1l[:, wof:wof + 128],
                                s * B)
                        idxt = ip.tile([128, B], I32, tag="ix")
                        nc.scalar.activation(idxt[:], ps[:], ACTF.Identity,
                                             bias=bck[:, S1 + s:S1 + s + 1],
                                             scale=bck[:, s:s + 1])
                        asl = acc1[:, fc * B:(fc + 1) * B]
                        if k == 0 and s == 0:
                            nc.vector.tensor_scalar(out=asl, in0=idxt[:],
                                                    scalar1=bck[:, 2 * S1 + s:2 * S1 + s + 1],
                                                    scalar2=0.0, op0=OP.mult, op1=OP.add)
                        else:
                            nc.vector.scalar_tensor_tensor(
                                out=asl, in0=idxt[:],
                                scalar=bck[:, 2 * S1 + s:2 * S1 + s + 1],
                                in1=asl, op0=OP.mult, op1=OP.add)

            # ---------------- layer-1 tail: sigma, dummy, row1 ----------------
            psg = psM.tile([1, KSL], F32, tag="m")
            nc.tensor.matmul(psg[:], sb_o7[:], sigc[:], start=True, stop=True)
            sgr = tp.tile([1, KSL], F32, tag="sg")
            nc.vector.tensor_copy(sgr[:], psg[:])
            sg = tp.tile([1, 1], F32, tag="sg1")
            nc.vector.tensor_reduce(out=sg[:], in_=sgr[:], axis=AX.X, op=OP.add)
            psdr = psM.tile([1, B], F32, tag="m")
            for h in range(BH):
                nc.tensor.matmul(psdr[:, h * NB:(h + 1) * NB], sb_o7[:],
                                 DqD[:, h * NB:(h + 1) * NB], start=True, stop=True)
            late = rp.tile([1, B], F32, tag="rowB")
            nc.vector.tensor_scalar(out=late[:], in0=psdr[:], scalar1=sb_cc[0:1, 4:5],
                                    scalar2=sg[:, 0:1], op0=OP.mult, op1=OP.add)
            late2 = rp.tile([1, B], F32, tag="rowB")
            nc.vector.tensor_tensor(out=late2[:], in0=late[:], in1=sb_row1[:], op=OP.add)
            plate = psM.tile([128, B], F32, tag="m")
            for h in range(BH):
                nc.tensor.matmul(plate[:, h * NB:(h + 1) * NB], sb_orow[:],
                                 late2[:, h * NB:(h + 1) * NB], start=True, stop=True)
            for fc in range(4):
                asl = acc1[:, fc * B:(fc + 1) * B]
                nc.vector.scalar_tensor_tensor(out=asl, in0=plate[:], scalar=1.0,
                                               in1=asl, op0=OP.mult, op1=OP.add)

            # ---------------- allreduce layer 1 ----------------
            ar_in = dp.tile([128, 4 * B], F32)
            ar_out = dp.tile([128, 4 * B], F32)
            nc.sync.dma_start(ar_in[:], acc1[:])
            nc.gpsimd.collective_compute(
                "AllReduce", OP.add, replica_groups=[list(range(8))],
                ins=[ar_in.opt()], outs=[ar_out.opt()])
            hsum = cp.tile([128, 4 * B], F32)
            nc.sync.dma_start(hsum[:], ar_out[:])

            # ---------------- tanh, hq, bits2 ----------------
            bits2 = cp.tile([128, 4 * B], BF16)
            hqbf = cp.tile([128, 4 * B], BF16)
            for fc in range(4):
                ht = wp.tile([128, B], F32, tag="w32")
                nc.scalar.activation(ht[:], hsum[:, fc * B:(fc + 1) * B], ACTF.Tanh,
                                     bias=0.0, scale=KAPPA)
                nc.sync.dma_start(hdbg[:, fc * B:(fc + 1) * B], ht[:])
                hc = wp.tile([128, B], F32, tag="w32")
                nc.vector.tensor_scalar(out=hc[:], in0=ht[:], scalar1=0.0, scalar2=1.0,
                                        op0=OP.max, op1=OP.min)
                hq = wp.tile([128, B], I32, tag="wi32")
                nc.vector.tensor_scalar(out=hq[:], in0=hc[:], scalar1=255.0, scalar2=0.0,
                                        op0=OP.mult, op1=OP.add)
                nc.vector.tensor_scalar(out=hqbf[:, fc * B:(fc + 1) * B], in0=hq[:],
                                        scalar1=1.0, scalar2=0.0, op0=OP.mult, op1=OP.add)
                bsh = wp.tile([128, B], I32, tag="wi32")
                nc.vector.tensor_scalar(out=bsh[:], in0=hq[:], scalar1=sb_cc[:, 5:6],
                                        scalar2=-0.499, op0=OP.mult, op1=OP.add)
                half = wp.tile([128, B], I32, tag="wi32")
                nc.vector.tensor_scalar(out=half[:], in0=bsh[:], scalar1=0.5,
                                        scalar2=-0.499, op0=OP.mult, op1=OP.add)
                nc.vector.scalar_tensor_tensor(out=bits2[:, fc * B:(fc + 1) * B],
                                               in0=half[:], scalar=-2.0, in1=bsh[:],
                                               op0=OP.mult, op1=OP.add)

            # hqsum row
            pshq = psM.tile([1, B], F32, tag="m")
            for fc in range(4):
                for h in range(BH):
                    nc.tensor.matmul(pshq[:, h * NB:(h + 1) * NB], sb_o128[:],
                                     hqbf[:, fc * B + h * NB: fc * B + (h + 1) * NB],
                                     start=(fc == 0), stop=(fc == 3))
            hrow = rp.tile([1, B], F32, tag="rowB")
            nc.vector.tensor_scalar(out=hrow[:], in0=pshq[:],
                                    scalar1=float(np.float32(-1.0 / (255.0 * KAPPA * 8.0))),
                                    scalar2=0.0, op0=OP.mult, op1=OP.add)

            # ---------------- dummy D2 ----------------
            psD2 = psM.tile([S2, B], F32, tag="m")
            for s in range(S2):
                for h in range(BH):
                    nc.tensor.matmul(psD2[:, h * NB:(h + 1) * NB],
                                     sb_e42[:, s * S2:(s + 1) * S2],
                                     bits2[:, s * B + h * NB: s * B + (h + 1) * NB],
                                     start=(s == 0), stop=(s == S2 - 1))
            D2sb = wp.tile([S2, B], F32, tag="w32")
            nc.vector.tensor_scalar(out=D2sb[:], in0=psD2[:], scalar1=C03, scalar2=0.0,
                                    op0=OP.mult, op1=OP.add)
            mxD2 = tp.tile([S2, 1], F32, tag="t2")
            mnD2 = tp.tile([S2, 1], F32, tag="t2")
            nc.vector.tensor_reduce(out=mxD2[:], in_=D2sb[:], axis=AX.X, op=OP.max)
            nc.vector.tensor_reduce(out=mnD2[:], in_=D2sb[:], axis=AX.X, op=OP.min)
            dD2 = tp.tile([S2, 1], F32, tag="t2")
            nc.vector.tensor_tensor(out=dD2[:], in0=mxD2[:], in1=mnD2[:], op=OP.subtract)
            rcD2 = tp.tile([S2, 1], F32, tag="t2")
            nc.vector.reciprocal(rcD2[:], dD2[:])
            mkD2 = tp.tile([S2, 1], F32, tag="t2")
            nc.vector.tensor_scalar(out=mkD2[:], in0=dD2[:], scalar1=0.0, scalar2=0.0,
                                    op0=OP.is_gt, op1=OP.add)
            rD2 = tp.tile([S2, 1], F32, tag="t2")
            nc.vector.tensor_scalar(out=rD2[:], in0=rcD2[:], scalar1=mkD2[:, 0:1],
                                    scalar2=RSCALE, op0=OP.mult, op1=OP.mult)
            rD2n = tp.tile([S2, 1], F32, tag="t2")
            nc.vector.tensor_scalar(out=rD2n[:], in0=rD2[:], scalar1=-1.0, scalar2=0.0,
                                    op0=OP.mult, op1=OP.add)
            bD2 = tp.tile([S2, 1], F32, tag="t2")
            nc.vector.tensor_scalar(out=bD2[:], in0=mnD2[:], scalar1=rD2n[:, 0:1],
                                    scalar2=-0.5, op0=OP.mult, op1=OP.add)
            stD2 = tp.tile([S2, 1], F32, tag="t2")
            nc.vector.tensor_scalar(out=stD2[:], in0=dD2[:], scalar1=STEPS, scalar2=0.0,
                                    op0=OP.mult, op1=OP.add)
            idxD2 = wp.tile([S2, B], I32, tag="wi32")
            nc.vector.tensor_scalar(out=idxD2[:], in0=D2sb[:], scalar1=rD2[:, 0:1],
                                    scalar2=bD2[:, 0:1], op0=OP.mult, op1=OP.add)
            DqD2 = wp.tile([S2, B], F32, tag="w32")
            nc.vector.tensor_scalar(out=DqD2[:], in0=idxD2[:], scalar1=stD2[:, 0:1],
                                    scalar2=mnD2[:, 0:1], op0=OP.mult, op1=OP.add)

            # ---------------- layer-2 main ----------------
            def emit_p2(ps, s2):
                wof = s2 * KSL * F2
                for h in range(BH):
                    nc.tensor.matmul(ps[:, h * NB:(h + 1) * NB],
                                     sb_g2h[:, wof:wof + KSL * F2],
                                     bits2[:, s2 * B + h * NB: s2 * B + (h + 1) * NB],
                                     start=True, stop=False)
                for h in range(BH):
                    nc.tensor.matmul(ps[:, h * NB:(h + 1) * NB],
                                     sb_g2l[:, wof:wof + KSL * F2],
                                     bits2[:, s2 * B + h * NB: s2 * B + (h + 1) * NB],
                                     start=False, stop=True)

            M2 = KSL * F2
            maxC2 = lp.tile([M2, 2 * S2], F32, tag="c2")
            for s2 in range(S2):
                ps = psA.tile([M2, B], F32, tag="p1")
                emit_p2(ps, s2)
                nc.vector.tensor_reduce(out=maxC2[:, s2:s2 + 1], in_=ps[:],
                                        axis=AX.X, op=OP.max)
                nc.vector.tensor_reduce(out=maxC2[:, S2 + s2:S2 + s2 + 1], in_=ps[:],
                                        axis=AX.X, op=OP.min)
            pt2a = psM.tile([S2, M2], F32, tag="m")
            nc.tensor.transpose(pt2a[:], maxC2[:, 0:S2], sb_id[0:M2, 0:M2])
            tca = tp.tile([S2, M2], F32, tag="tcc")
            nc.vector.tensor_copy(tca[:], pt2a[:])
            pt2b = psM.tile([S2, M2], F32, tag="m")
            nc.tensor.transpose(pt2b[:], maxC2[:, S2:2 * S2], sb_id[0:M2, 0:M2])
            tcb = tp.tile([S2, M2], F32, tag="tcc")
            nc.vector.tensor_copy(tcb[:], pt2b[:])
            mx2 = tp.tile([S2, KSL], F32, tag="q")
            mn2 = tp.tile([S2, KSL], F32, tag="q")
            nc.vector.tensor_reduce(out=mx2[:],
                                    in_=tca[:].rearrange("p (k f) -> p k f", f=F2),
                                    axis=AX.X, op=OP.max)
            nc.vector.tensor_reduce(out=mn2[:],
                                    in_=tcb[:].rearrange("p (k f) -> p k f", f=F2),
                                    axis=AX.X, op=OP.min)
            d2 = tp.tile([S2, KSL], F32, tag="q")
            nc.vector.tensor_tensor(out=d2[:], in0=mx2[:], in1=mn2[:], op=OP.subtract)
            rc2 = tp.tile([S2, KSL], F32, tag="q")
            nc.vector.reciprocal(rc2[:], d2[:])
            mk2 = tp.tile([S2, KSL], F32, tag="q")
            nc.vector.tensor_scalar(out=mk2[:], in0=d2[:], scalar1=0.0, scalar2=0.0,
                                    op0=OP.is_gt, op1=OP.add)
            r2t = tp.tile([S2, KSL], F32, tag="q")
            nc.vector.tensor_tensor(out=r2t[:], in0=rc2[:], in1=mk2[:], op=OP.mult)
            nc.vector.tensor_scalar(out=r2t[:], in0=r2t[:], scalar1=RSCALE, scalar2=0.0,
                                    op0=OP.mult, op1=OP.add)
            b2t = tp.tile([S2, KSL], F32, tag="q")
            nc.vector.tensor_tensor(out=b2t[:], in0=mn2[:], in1=r2t[:], op=OP.mult)
            nc.vector.tensor_scalar(out=b2t[:], in0=b2t[:], scalar1=-1.0, scalar2=-0.5,
                                    op0=OP.mult, op1=OP.add)
            st2t = tp.tile([S2, KSL], F32, tag="q")
            nc.vector.tensor_scalar(out=st2t[:], in0=d2[:], scalar1=STEPS, scalar2=0.0,
                                    op0=OP.mult, op1=OP.add)
            c2t = tp.tile([S2, KSL], F32, tag="q")
            nc.vector.tensor_tensor(out=c2t[:], in0=st2t[:], in1=sb_s2t[:], op=OP.mult)
            smn2 = tp.tile([S2, KSL], F32, tag="q")
            nc.vector.tensor_tensor(out=smn2[:], in0=mn2[:], in1=sb_s2t[:], op=OP.mult)
            psg2 = psM.tile([1, KSL], F32, tag="m")
            nc.tensor.matmul(psg2[:], sb_o7[0:S2, :], smn2[:], start=True, stop=True)
            sg2r = tp.tile([1, KSL], F32, tag="sg")
            nc.vector.tensor_copy(sg2r[:], psg2[:])
            sg2 = tp.tile([1, 1], F32, tag="sg1")
            nc.vector.tensor_reduce(out=sg2[:], in_=sg2r[:], axis=AX.X, op=OP.add)
            # transpose r2/b2/c2 ([s2,k] -> [k,s2]) via PE, then per-s2 bcast
            psT = psM.tile([S2, 3 * KSL], F32, tag="m")
            nc.tensor.transpose(psT[:, 0:KSL], r2t[:], sb_id[0:S2, 0:S2])
            nc.tensor.transpose(psT[:, KSL:2 * KSL], b2t[:], sb_id[0:S2, 0:S2])
            nc.tensor.transpose(psT[:, 2 * KSL:3 * KSL], c2t[:], sb_id[0:S2, 0:S2])
            sT = tp.tile([S2, 3 * KSL], F32, tag="sT")
            nc.vector.tensor_copy(sT[:], psT[:])
            scl2 = []
            for s2 in range(S2):
                rhs = tp.tile([KSL, 3], F32, tag="rh")
                nc.vector.tensor_copy(rhs[:, 0:1], sT[:, s2:s2 + 1])
                nc.vector.tensor_copy(rhs[:, 1:2], sT[:, KSL + s2:KSL + s2 + 1])
                nc.vector.tensor_copy(rhs[:, 2:3], sT[:, 2 * KSL + s2:2 * KSL + s2 + 1])
                psc = psM.tile([M2, 3], F32, tag="m")
                nc.tensor.matmul(psc[:], sb_bind[:], rhs[:], start=True, stop=True)
                sc = tp.tile([M2, 3], F32, tag="sc%d" % s2)
                nc.vector.tensor_copy(sc[:], psc[:])
                scl2.append(sc)
            acc2 = cp.tile([M2, B], F32)
            for s2 in range(S2):
                ps = psB.tile([M2, B], F32, tag="p2")
                emit_p2(ps, s2)
                idxt = ip.tile([M2, B], I32, tag="ix")
                nc.scalar.activation(idxt[:], ps[:], ACTF.Identity,
                                     bias=scl2[s2][:, 1:2], scale=scl2[s2][:, 0:1])
                if s2 == 0:
                    nc.vector.tensor_scalar(out=acc2[:], in0=idxt[:],
                                            scalar1=scl2[s2][:, 2:3], scalar2=0.0,
                                            op0=OP.mult, op1=OP.add)
                else:
                    nc.vector.scalar_tensor_tensor(out=acc2[:], in0=idxt[:],
                                                   scalar=scl2[s2][:, 2:3], in1=acc2[:],
                                                   op0=OP.mult, op1=OP.add)
            # dummy2 row + sigma2 + hq row -> late2row
            psd2 = psM.tile([1, B], F32, tag="m")
            for h in range(BH):
                nc.tensor.matmul(psd2[:, h * NB:(h + 1) * NB], sb_o7[0:S2, :],
                                 DqD2[:, h * NB:(h + 1) * NB], start=True, stop=True)
            l2a = rp.tile([1, B], F32, tag="rowB")
            nc.vector.tensor_scalar(out=l2a[:], in0=psd2[:], scalar1=sb_cc[0:1, 4:5],
                                    scalar2=sg2[:, 0:1], op0=OP.mult, op1=OP.add)
            l2b = rp.tile([1, B], F32, tag="rowB")
            nc.vector.tensor_tensor(out=l2b[:], in0=l2a[:], in1=hrow[:], op=OP.add)
            # fold 40 -> 10 plus rank-1 late row
            psf = psM.tile([F2, B], F32, tag="m")
            for h in range(BH):
                nc.tensor.matmul(psf[:, h * NB:(h + 1) * NB], sb_f40[:],
                                 acc2[:, h * NB:(h + 1) * NB], start=True, stop=False)
            for h in range(BH):
                nc.tensor.matmul(psf[:, h * NB:(h + 1) * NB], sb_orow[0:1, 0:F2],
                                 l2b[:, h * NB:(h + 1) * NB], start=False, stop=True)
            o2a = rp.tile([F2, B], F32, tag="rowB")
            nc.vector.tensor_copy(o2a[:], psf[:])
            ar2i = dp.tile([F2, B], F32)
            ar2o = dp.tile([F2, B], F32)
            nc.sync.dma_start(ar2i[:], o2a[:])
            nc.gpsimd.collective_compute(
                "AllReduce", OP.add, replica_groups=[list(range(8))],
                ins=[ar2i.opt()], outs=[ar2o.opt()])
            o2b = rp.tile([F2, B], F32, tag="rowB")
            nc.sync.dma_start(o2b[:], ar2o[:])
            o2c = rp.tile([F2, B], F32, tag="rowB")
            nc.vector.tensor_scalar(out=o2c[:], in0=o2b[:], scalar1=KAPPA, scalar2=0.0,
                                    op0=OP.mult, op1=OP.add)
            nc.sync.dma_start(out2d[:], o2c[:])

    nc.compile()
    _NC_CACHE[B] = nc
    return nc


# ------------------------------------------------------------------ driver
def run_cores(inputs, B=1024, want_debug=False, trace=False):
    shared, per_core = host_prepare(inputs["x"], inputs["w1"], inputs["w3"],
                                    inputs["noise1"], inputs["noise3"], B)
    nc = build_nc(B)
    in_maps = [{**shared, **pc} for pc in per_core]
    res = bass_utils.run_bass_kernel_spmd(nc, in_maps, list(range(8)), trace=trace)
    out = np.asarray(res.results[0]["out2"]).T.astype(np.float32)
    if want_debug:
        return out, res
    return out


def kernel(**inputs):
    return run_cores(inputs, B=1024)



# revision 20
# speedup vs baseline: 3880.9122x; 3880.9122x over previous
"""Trainium2 Bass kernel for nn_DFANet (analog PIM crossbar MLP emulation).

Sharding: input-bit-plane parallel — core c owns input bit i=c for layer 1
and hq bit i2=c for layer 2. All ADC min/max groups are then core-local;
one fp32 sum-AllReduce of the accumulator happens at each layer boundary.

Self-contained: hardcodes all shapes; host precomputes bit-planes and
conductance tensors (exact fp32 mirror of the reference formulas, split
into bf16 hi+lo pairs so PE products with 0/1 bits are fp32-grade).

Execution path: a cached jax.jit(shard_map) over the bass_exec custom
call (the same machinery bass_utils.run_bass_kernel_spmd uses under
axon), with all input tensors device-resident so repeated calls ship no
data. build_nc(nrep=N) unrolls the whole program N times inside one NEFF
for steady-state per-iteration timing.
"""
import math
import sys

import numpy as np

sys.path.insert(0, "/opt/trn_rl_repo")

import ml_dtypes  # noqa: E402
import jax  # noqa: E402
from jax.sharding import Mesh, PartitionSpec, NamedSharding  # noqa: E402
from jax.experimental.shard_map import shard_map  # noqa: E402

import concourse.bass as bass  # noqa: E402
import concourse.mybir as mybir  # noqa: E402
import concourse.tile as tile  # noqa: E402
import concourse.bacc as bacc  # noqa: E402
from concourse import bass_utils  # noqa: E402
from concourse.bass2jax import (  # noqa: E402
    _bass_exec_p,
    partition_id_tensor,
    install_neuronx_cc_hook,
)

F32 = mybir.dt.float32
BF16 = mybir.dt.bfloat16
I32 = mybir.dt.int32
AX = mybir.AxisListType
OP = mybir.AluOpType
ACTF = mybir.ActivationFunctionType

# problem constants
I_BITS = 8
S1, S2 = 7, 4
KSL = 4            # weight slices
F1, F2 = 512, 10
N1, N2 = 784, 512
NP1 = S1 * 128
CR = 4.0
LOWER, UPPER = np.float32(1.0 / 10.0), np.float32(1.0)
GLO = np.float32(np.float32(CR - 1.0) * LOWER)      # (cr-1)*lower
GSC = np.float32(UPPER - LOWER)                     # 0.9
KAPPA = float(np.float32(2.0 / (0.9 * 255.0 * 255.0)))
RSCALE = float(np.float32(32.0 * (1.0 - 2.0 ** -22)))
STEPS = float(np.float32(2.0 ** -5))
C03 = float(np.float32(3.0) * np.float32(0.1))      # (cr-1)*lower as f32

N_CORES = 8

_NC_CACHE = {}
_RUNNER_CACHE = {}


# ----------------------------------------------------------------- host prep
def _qweights(w):
    """Xi -> slices -> conductances, mirroring reference fp32 ops exactly."""
    w = np.asarray(w, np.float32)
    Xi = np.clip(np.round((w + np.float32(1.0)) * np.float32(0.5) * np.float32(255.0)),
                 0.0, 255.0).astype(np.float32)
    return Xi


def _gtensor(Xi, noise, S):
    """g[f, s*128+a, k] fp32, padded to S*128 rows; mirrors reference."""
    F, N = Xi.shape
    Np = S * 128
    Xi = np.pad(Xi, ((0, 0), (0, Np - N)))
    kpow = (np.float32(CR) ** np.arange(KSL)).astype(np.float32)
    slc = np.mod(np.floor(Xi[..., None] / kpow), np.float32(CR)).astype(np.float32)
    g = slc * GSC + GLO
    g = (g * (np.float32(1.0) + np.float32(0.05) * np.asarray(noise, np.float32))).astype(np.float32)
    return g  # [F, Np, K]


def _hi_lo(x):
    hi = x.astype(ml_dtypes.bfloat16)
    lo = (x - hi.astype(np.float32)).astype(ml_dtypes.bfloat16)
    return hi, lo


def host_prepare(x, w1, w3, noise1, noise3, B):
    """Returns (shared dict, per-core list of dicts) of DRAM input arrays."""
    x = np.asarray(x, np.float32)[:B]
    xq = np.round(np.clip(x, 0.0, 1.0) * np.float32(255.0)).astype(np.float32)  # [B, N1]
    xq_pad = np.pad(xq, ((0, 0), (0, NP1 - N1)))
    zpow = (np.float32(2.0) ** np.arange(I_BITS)).astype(np.float32)
    bits = np.mod(np.floor(xq_pad[..., None] / zpow), np.float32(2.0))  # [B, NP1, I]
    # bitsT per i: [128, S1*B] bf16, block s cols = bits[:, s*128+a, i].T
    bitsT = np.transpose(bits, (2, 1, 0))  # [I, NP1, B]
    bitsT = bitsT.reshape(I_BITS, S1, 128, B)

    g1 = _gtensor(_qweights(w1), noise1, S1)          # [512, 896, 4]
    # lhsT layout per (k,s): [a=128, f=512]; slot sk = k*7+s
    g1l = np.transpose(g1.reshape(F1, S1, 128, KSL), (3, 1, 2, 0))  # [K, S1, 128, F1]
    g1flat = g1l.reshape(KSL * S1, 128, F1).transpose(1, 0, 2).reshape(128, KSL * S1 * F1)
    g1hi, g1lo = _hi_lo(np.ascontiguousarray(g1flat))

    g2 = _gtensor(_qweights(w3), noise3, S2)          # [10, 512, 4]
    # per s2: [a=128, 40] with col k*10+f
    g2l = np.transpose(g2.reshape(F2, S2, 128, KSL), (1, 2, 3, 0))  # [S2, 128, K, F2]
    g2flat = g2l.reshape(S2, 128, KSL * F2).transpose(1, 0, 2).reshape(128, S2 * KSL * F2)
    g2hi, g2lo = _hi_lo(np.ascontiguousarray(g2flat))

    xqsum = xq.sum(axis=1, dtype=np.float32).astype(np.float32)   # [B]
    row1 = (-(xqsum / np.float32(255.0)) / np.float32(KAPPA) / np.float32(8.0)
            ).astype(np.float32)[None, :]  # [1, B]

    e7 = np.zeros((128, S1 * S1), np.float32)
    for s in range(S1):
        e7[:, s * S1 + s] = 1.0
    e42 = np.zeros((128, S2 * S2), np.float32)
    for s in range(S2):
        e42[:, s * S2 + s] = 1.0
    ones128 = np.ones((128, 1), np.float32)
    onesrow = np.ones((1, 128), np.float32)
    ones7 = np.ones((S1, 1), np.float32)
    fold40 = np.zeros((KSL * F2, F2), np.float32)
    for k in range(KSL):
        for j in range(F2):
            fold40[k * F2 + j, j] = 1.0
    blockind = np.zeros((KSL, KSL * F2), np.float32)
    for k in range(KSL):
        blockind[k, k * F2:(k + 1) * F2] = 1.0
    ident = np.eye(128, dtype=np.float32)
    scal2t = np.zeros((S2, KSL), np.float32)

    shared = dict(
        g1hi=np.asarray(g1hi), g1lo=np.asarray(g1lo),
        g2hi=np.asarray(g2hi), g2lo=np.asarray(g2lo),
        e7=e7.astype(ml_dtypes.bfloat16), e42=e42.astype(ml_dtypes.bfloat16),
        ones128=ones128.astype(ml_dtypes.bfloat16),
        onesrow=onesrow, ones7=ones7, fold40=fold40, blockind=blockind,
        ident=ident, row1=row1,
    )
    per_core = []
    for c in range(8):
        sc = np.float32(2.0 ** c)
        cconst = np.zeros((128, 16), np.float32)
        for k in range(KSL):
            cconst[:, k] = sc * np.float32(4.0 ** k)
        cconst[:, 4] = -np.float32(85.0) * sc
        cconst[:, 5] = np.float32(2.0 ** -c)
        st2 = scal2t.copy()
        for k in range(KSL):
            st2[:, k] = sc * np.float32(4.0 ** k)
        btc = np.ascontiguousarray(
            bitsT[c].transpose(1, 0, 2).reshape(128, S1 * B)).astype(ml_dtypes.bfloat16)
        per_core.append(dict(bitsT=np.asarray(btc), cconst=cconst, scal2t=st2))
    return shared, per_core


# ------------------------------------------------------------- bass program
def build_nc(B, nrep=1, debug=False, store_p=0, gp_accum=False,
             abl_nop2mm=False, abl_noar=False, gp_minred=False, gp_max=False,
             ar_chunk=False):
    key = (B, nrep, debug, store_p, gp_accum, abl_nop2mm, abl_noar, gp_minred,
           gp_max, ar_chunk)
    if key in _NC_CACHE:
        return _NC_CACHE[key]
    BH = B // 512 if B >= 512 else 1
    NB = min(B, 512)                      # matmul moving chunk
    nc = bacc.Bacc("TRN2", target_bir_lowering=False, debug=False,
                   num_devices=8)

    def din(name, shape, dt):
        return nc.dram_tensor(name, list(shape), dt, kind="ExternalInput")[:]

    bitsT = din("bitsT", (128, S1 * B), BF16)
    g1hi = din("g1hi", (128, KSL * S1 * F1), BF16)
    g1lo = din("g1lo", (128, KSL * S1 * F1), BF16)
    g2hi = din("g2hi", (128, S2 * KSL * F2), BF16)
    g2lo = din("g2lo", (128, S2 * KSL * F2), BF16)
    e7 = din("e7", (128, S1 * S1), BF16)
    e42 = din("e42", (128, S2 * S2), BF16)
    ones128 = din("ones128", (128, 1), BF16)
    onesrow = din("onesrow", (1, 128), F32)
    ones7 = din("ones7", (S1, 1), F32)
    fold40 = din("fold40", (KSL * F2, F2), F32)
    blockind = din("blockind", (KSL, KSL * F2), F32)
    ident = din("ident", (128, 128), F32)
    row1 = din("row1", (1, B), F32)
    cconst = din("cconst", (128, 16), F32)
    scal2t = din("scal2t", (S2, KSL), F32)

    out2d = nc.dram_tensor("out2", [F2, B], F32, kind="ExternalOutput")[:]
    hdbg = (nc.dram_tensor("h_dbg", [128, 4 * B], F32, kind="ExternalOutput")[:]
            if debug else None)

    with tile.TileContext(nc) as tc:
        with (
            tc.tile_pool(name="const", bufs=1) as cp,
            tc.tile_pool(name="work", bufs=4) as wp,
            tc.tile_pool(name="idx", bufs=3) as ip,
            tc.tile_pool(name="tiny", bufs=16) as tp,
            tc.tile_pool(name="coll", bufs=2) as lp,
            tc.tile_pool(name="rows", bufs=3) as rp,
            tc.tile_pool(name="pstore", bufs=1) as pp,
            tc.tile_pool(name="ps_a", bufs=2, space="PSUM") as psA,
            tc.tile_pool(name="ps_b", bufs=1, space="PSUM") as psB,
            tc.tile_pool(name="ps_m", bufs=1, space="PSUM") as psM,
            tc.tile_pool(name="dram", bufs=1, space="DRAM") as dp,
        ):
          for _rep in range(nrep):
            # ---- load constants to SBUF
            def load(ap, shape, dt, tag):
                t = cp.tile(list(shape), dt, tag=tag)
                nc.sync.dma_start(t[:], ap)
                return t

            sb_bits = load(bitsT, (128, S1 * B), BF16, tag="sb_bits")
            sb_g1h = load(g1hi, (128, KSL * S1 * F1), BF16, tag="sb_g1h")
            sb_g1l = load(g1lo, (128, KSL * S1 * F1), BF16, tag="sb_g1l")
            sb_g2h = load(g2hi, (128, S2 * KSL * F2), BF16, tag="sb_g2h")
            sb_g2l = load(g2lo, (128, S2 * KSL * F2), BF16, tag="sb_g2l")
            sb_e7 = load(e7, (128, S1 * S1), BF16, tag="sb_e7")
            sb_e42 = load(e42, (128, S2 * S2), BF16, tag="sb_e42")
            sb_o128 = load(ones128, (128, 1), BF16, tag="sb_o128")
            sb_orow = load(onesrow, (1, 128), F32, tag="sb_orow")
            sb_o7 = load(ones7, (S1, 1), F32, tag="sb_o7")
            sb_f40 = load(fold40, (KSL * F2, F2), F32, tag="sb_f40")
            sb_bind = load(blockind, (KSL, KSL * F2), F32, tag="sb_bind")
            sb_id = load(ident, (128, 128), F32, tag="sb_id")
            sb_row1 = load(row1, (1, B), F32, tag="sb_row1")
            sb_cc = load(cconst, (128, 16), F32, tag="sb_cc")
            sb_s2t = load(scal2t, (S2, KSL), F32, tag="sb_s2t")

            acc1 = cp.tile([128, 4 * B], F32, tag="acc1")
            sigc = cp.tile([S1, KSL], F32, tag="sigc")

            # PE warm-up on every DMA-loaded constant it will read later, so
            # later matmuls don't need a third (DMA) sync-wait slot.
            warm = psM.tile([1, 16], F32, tag="m")
            for j, t in enumerate([sb_id, sb_bind, sb_f40, sb_orow, sb_o7,
                                   sb_g1h, sb_g1l, sb_g2h, sb_g2l, sb_e7,
                                   sb_e42, sb_o128, sb_bits, sb_row1]):
                nc.tensor.matmul(warm[0:1, j:j + 1], t[0:1, 0:1], t[0:1, 0:1],
                                 start=True, stop=True)

            def emit_p1(ps, wslice_hi, wslice_lo, rhs_base):
                for h in range(BH):
                    nc.tensor.matmul(ps[:, h * NB:(h + 1) * NB], wslice_hi,
                                     sb_bits[:, rhs_base + h * NB: rhs_base + (h + 1) * NB],
                                     start=True, stop=False)
                for h in range(BH):
                    nc.tensor.matmul(ps[:, h * NB:(h + 1) * NB], wslice_lo,
                                     sb_bits[:, rhs_base + h * NB: rhs_base + (h + 1) * NB],
                                     start=False, stop=True)

            # ---------------- dummy D1 ----------------
            psD = psM.tile([S1, B], F32, tag="m")
            for s in range(S1):
                for h in range(BH):
                    nc.tensor.matmul(psD[:, h * NB:(h + 1) * NB],
                                     sb_e7[:, s * S1:(s + 1) * S1],
                                     sb_bits[:, s * B + h * NB: s * B + (h + 1) * NB],
                                     start=(s == 0), stop=(s == S1 - 1))
            Dsb = wp.tile([S1, B], F32, tag="w32")
            nc.vector.tensor_scalar(out=Dsb[:], in0=psD[:], scalar1=C03, scalar2=0.0,
                                    op0=OP.mult, op1=OP.add)
            mxD = tp.tile([S1, 1], F32, tag="t")
            mnD = tp.tile([S1, 1], F32, tag="t")
            nc.vector.tensor_reduce(out=mxD[:], in_=Dsb[:], axis=AX.X, op=OP.max)
            nc.vector.tensor_reduce(out=mnD[:], in_=Dsb[:], axis=AX.X, op=OP.min)
            dD = tp.tile([S1, 1], F32, tag="t")
            nc.vector.tensor_tensor(out=dD[:], in0=mxD[:], in1=mnD[:], op=OP.subtract)
            rcD = tp.tile([S1, 1], F32, tag="t")
            nc.vector.reciprocal(rcD[:], dD[:])
            mkD = tp.tile([S1, 1], F32, tag="t")
            nc.vector.tensor_scalar(out=mkD[:], in0=dD[:], scalar1=0.0, scalar2=0.0,
                                    op0=OP.is_gt, op1=OP.add)
            rD = tp.tile([S1, 1], F32, tag="t")
            nc.vector.tensor_scalar(out=rD[:], in0=rcD[:], scalar1=mkD[:, 0:1],
                                    scalar2=RSCALE, op0=OP.mult, op1=OP.mult)
            rDn = tp.tile([S1, 1], F32, tag="t")
            nc.vector.tensor_scalar(out=rDn[:], in0=rD[:], scalar1=-1.0, scalar2=0.0,
                                    op0=OP.mult, op1=OP.add)
            bD = tp.tile([S1, 1], F32, tag="t")
            nc.vector.tensor_scalar(out=bD[:], in0=mnD[:], scalar1=rDn[:, 0:1],
                                    scalar2=-0.5, op0=OP.mult, op1=OP.add)
            stD = tp.tile([S1, 1], F32, tag="t")
            nc.vector.tensor_scalar(out=stD[:], in0=dD[:], scalar1=STEPS, scalar2=0.0,
                                    op0=OP.mult, op1=OP.add)
            idxD = wp.tile([S1, B], I32, tag="wi32")
            nc.vector.tensor_scalar(out=idxD[:], in0=Dsb[:], scalar1=rD[:, 0:1],
                                    scalar2=bD[:, 0:1], op0=OP.mult, op1=OP.add)
            DqD = wp.tile([S1, B], F32, tag="w32")
            nc.vector.tensor_scalar(out=DqD[:], in0=idxD[:], scalar1=stD[:, 0:1],
                                    scalar2=mnD[:, 0:1], op0=OP.mult, op1=OP.add)

            # ---------------- layer-1 main loop ----------------
            for k in range(KSL):
                maxC = lp.tile([128, S1 * KSL], F32, tag="mx")
                minC = lp.tile([128, S1 * KSL], F32, tag="mn")
                gmx = lp.tile([1, S1 * KSL], F32, tag="gmx")
                pstore = {}
                # pass 1
                for s in range(S1):
                    for fc in range(4):
                        ps = psA.tile([128, B], F32, tag="p1")
                        wof = (k * S1 + s) * F1 + fc * 128
                        emit_p1(ps, sb_g1h[:, wof:wof + 128], sb_g1l[:, wof:wof + 128],
                                s * B)
                        if fc < store_p:
                            sbP = pp.tile([128, B], F32, tag="P%d_%d" % (s, fc))
                            nc.scalar.activation(sbP[:], ps[:], ACTF.Identity,
                                                 bias=0.0, scale=1.0)
                            pstore[(s, fc)] = sbP
                        if gp_max:
                            nc.gpsimd.tensor_reduce(
                                out=gmx[0:1, s * 4 + fc:s * 4 + fc + 1],
                                in_=ps[:], axis=AX.XYZWC, op=OP.max)
                        else:
                            nc.vector.tensor_reduce(out=maxC[:, s * 4 + fc:s * 4 + fc + 1],
                                                    in_=ps[:], axis=AX.X, op=OP.max)
                        nc.vector.tensor_reduce(out=minC[:, s * 4 + fc:s * 4 + fc + 1],
                                                in_=ps[:], axis=AX.X, op=OP.min)
                # combine k: fc-fold then transpose then partition fold
                red = tp.tile([128, 2 * S1], F32, tag="red")
                if not gp_max:
                    nc.vector.tensor_reduce(out=red[:, 0:S1],
                                            in_=maxC[:].rearrange("p (s f) -> p s f", f=4),
                                            axis=AX.X, op=OP.max)
                nc.vector.tensor_reduce(out=red[:, S1:2 * S1],
                                        in_=minC[:].rearrange("p (s f) -> p s f", f=4),
                                        axis=AX.X, op=OP.min)
                mx = tp.tile([S1, 1], F32, tag="t")
                if gp_max:
                    redr = tp.tile([1, S1], F32, tag="redr")
                    nc.vector.tensor_reduce(
                        out=redr[:], in_=gmx[:].rearrange("p (s f) -> p s f", f=4),
                        axis=AX.X, op=OP.max)
                    pmx = psM.tile([S1, 1], F32, tag="m")
                    nc.tensor.transpose(pmx[:], redr[:], sb_id[0:1, 0:1])
                    nc.vector.tensor_copy(mx[:], pmx[:])
                else:
                    ptm = psM.tile([S1, 128], F32, tag="m")
                    nc.tensor.transpose(ptm[:], red[:, 0:S1], sb_id[:])
                    tcm = tp.tile([S1, 128], F32, tag="tc")
                    nc.vector.tensor_copy(tcm[:], ptm[:])
                    nc.vector.tensor_reduce(out=mx[:], in_=tcm[:], axis=AX.X, op=OP.max)
                ptn = psM.tile([S1, 128], F32, tag="m")
                nc.tensor.transpose(ptn[:], red[:, S1:2 * S1], sb_id[:])
                tcn = tp.tile([S1, 128], F32, tag="tc")
                nc.vector.tensor_copy(tcn[:], ptn[:])
                mn = tp.tile([S1, 1], F32, tag="t")
                nc.vector.tensor_reduce(out=mn[:], in_=tcn[:], axis=AX.X, op=OP.min)
                d = tp.tile([S1, 1], F32, tag="t")
                nc.vector.tensor_tensor(out=d[:], in0=mx[:], in1=mn[:], op=OP.subtract)
                rc = tp.tile([S1, 1], F32, tag="t")
                nc.vector.reciprocal(rc[:], d[:])
                mk = tp.tile([S1, 1], F32, tag="t")
                nc.vector.tensor_scalar(out=mk[:], in0=d[:], scalar1=0.0, scalar2=0.0,
                                        op0=OP.is_gt, op1=OP.add)
                rr = tp.tile([S1, 1], F32, tag="t")
                nc.vector.tensor_scalar(out=rr[:], in0=rc[:], scalar1=mk[:, 0:1],
                                        scalar2=RSCALE, op0=OP.mult, op1=OP.mult)
                rrn = tp.tile([S1, 1], F32, tag="t")
                nc.vector.tensor_scalar(out=rrn[:], in0=rr[:], scalar1=-1.0, scalar2=0.0,
                                        op0=OP.mult, op1=OP.add)
                bb = tp.tile([S1, 1], F32, tag="t")
                nc.vector.tensor_scalar(out=bb[:], in0=mn[:], scalar1=rrn[:, 0:1],
                                        scalar2=-0.5, op0=OP.mult, op1=OP.add)
                stp = tp.tile([S1, 1], F32, tag="t")
                nc.vector.tensor_scalar(out=stp[:], in0=d[:], scalar1=STEPS, scalar2=0.0,
                                        op0=OP.mult, op1=OP.add)
                cc = tp.tile([S1, 1], F32, tag="t")
                nc.vector.tensor_scalar(out=cc[:], in0=stp[:], scalar1=sb_cc[0:S1, k:k + 1],
                                        scalar2=0.0, op0=OP.mult, op1=OP.add)
                nc.vector.tensor_scalar(out=sigc[:, k:k + 1], in0=mn[:],
                                        scalar1=sb_cc[0:S1, k:k + 1], scalar2=0.0,
                                        op0=OP.mult, op1=OP.add)
                # broadcast r/b/c to [128, 3*S1]: transpose cols to one row, rank-1
                prow = psM.tile([1, 3 * S1], F32, tag="m")
                nc.tensor.transpose(prow[:, 0:S1], rr[:], sb_id[0:S1, 0:S1])
                nc.tensor.transpose(prow[:, S1:2 * S1], bb[:], sb_id[0:S1, 0:S1])
                nc.tensor.transpose(prow[:, 2 * S1:3 * S1], cc[:], sb_id[0:S1, 0:S1])
                row21 = tp.tile([1, 3 * S1], F32, tag="r21")
                nc.vector.tensor_copy(row21[:], prow[:])
                pbc = psM.tile([128, 3 * S1], F32, tag="m")
                nc.tensor.matmul(pbc[:], sb_orow[:], row21[:], start=True, stop=True)
                bck = tp.tile([128, 3 * S1], F32, tag="bck")
                nc.vector.tensor_copy(bck[:], pbc[:])
                # pass 2
                acc_eng = nc.gpsimd if gp_accum else nc.vector
                prev_ps = [None]
                for s in range(S1):
                    for fc in range(4):
                        if (s, fc) in pstore:
                            psrc = pstore[(s, fc)]
                        elif abl_nop2mm and prev_ps[0] is not None:
                            psrc = prev_ps[0]   # timing ablation only: wrong data
                        else:
                            ps = psB.tile([128, B], F32, tag="p2")
                            wof = (k * S1 + s) * F1 + fc * 128
                            emit_p1(ps, sb_g1h[:, wof:wof + 128],
                                    sb_g1l[:, wof:wof + 128], s * B)
                            psrc = ps
                            prev_ps[0] = ps
                        idxt = ip.tile([128, B], I32, tag="ix")
                        nc.scalar.activation(idxt[:], psrc[:], ACTF.Identity,
                                             bias=bck[:, S1 + s:S1 + s + 1],
                                             scale=bck[:, s:s + 1])
                        asl = acc1[:, fc * B:(fc + 1) * B]
                        if k == 0 and s == 0:
                            acc_eng.tensor_scalar(out=asl, in0=idxt[:],
                                                  scalar1=bck[:, 2 * S1 + s:2 * S1 + s + 1],
                                                  scalar2=0.0, op0=OP.mult, op1=OP.add)
                        else:
                            acc_eng.scalar_tensor_tensor(
                                out=asl, in0=idxt[:],
                                scalar=bck[:, 2 * S1 + s:2 * S1 + s + 1],
                                in1=asl, op0=OP.mult, op1=OP.add)

            # ---------------- layer-1 tail: sigma, dummy, row1 ----------------
            psg = psM.tile([1, KSL], F32, tag="m")
            nc.tensor.matmul(psg[:], sb_o7[:], sigc[:], start=True, stop=True)
            sgr = tp.tile([1, KSL], F32, tag="sg")
            nc.vector.tensor_copy(sgr[:], psg[:])
            sg = tp.tile([1, 1], F32, tag="sg1")
            nc.vector.tensor_reduce(out=sg[:], in_=sgr[:], axis=AX.X, op=OP.add)
            psdr = psM.tile([1, B], F32, tag="m")
            for h in range(BH):
                nc.tensor.matmul(psdr[:, h * NB:(h + 1) * NB], sb_o7[:],
                                 DqD[:, h * NB:(h + 1) * NB], start=True, stop=True)
            late = rp.tile([1, B], F32, tag="rowB")
            nc.vector.tensor_scalar(out=late[:], in0=psdr[:], scalar1=sb_cc[0:1, 4:5],
                                    scalar2=sg[:, 0:1], op0=OP.mult, op1=OP.add)
            late2 = rp.tile([1, B], F32, tag="rowB")
            nc.vector.tensor_tensor(out=late2[:], in0=late[:], in1=sb_row1[:], op=OP.add)
            plate = psM.tile([128, B], F32, tag="m")
            for h in range(BH):
                nc.tensor.matmul(plate[:, h * NB:(h + 1) * NB], sb_orow[:],
                                 late2[:, h * NB:(h + 1) * NB], start=True, stop=True)
            hsum = cp.tile([128, 4 * B], F32, tag="hsum")
            if ar_chunk:
                # per-fc boundary pipeline: accumulate tail, AR chunk, fetch
                for fc in range(4):
                    asl = acc1[:, fc * B:(fc + 1) * B]
                    nc.vector.scalar_tensor_tensor(out=asl, in0=plate[:], scalar=1.0,
                                                   in1=asl, op0=OP.mult, op1=OP.add)
                    ari = dp.tile([128, B], F32, tag="ar_in%d" % fc)
                    aro = dp.tile([128, B], F32, tag="ar_out%d" % fc)
                    nc.sync.dma_start(ari[:], asl)
                    if abl_noar:
                        nc.sync.dma_start(aro[:], ari[:])
                    else:
                        nc.gpsimd.collective_compute(
                            "AllReduce", OP.add, replica_groups=[list(range(8))],
                            ins=[ari.opt()], outs=[aro.opt()])
                    nc.sync.dma_start(hsum[:, fc * B:(fc + 1) * B], aro[:])
            else:
                for fc in range(4):
                    asl = acc1[:, fc * B:(fc + 1) * B]
                    nc.vector.scalar_tensor_tensor(out=asl, in0=plate[:], scalar=1.0,
                                                   in1=asl, op0=OP.mult, op1=OP.add)

                # ---------------- allreduce layer 1 ----------------
                ar_in = dp.tile([128, 4 * B], F32, tag="ar_in")
                ar_out = dp.tile([128, 4 * B], F32, tag="ar_out")
                nc.sync.dma_start(ar_in[:], acc1[:])
                if abl_noar:
                    nc.sync.dma_start(ar_out[:], ar_in[:])
                else:
                    nc.gpsimd.collective_compute(
                        "AllReduce", OP.add, replica_groups=[list(range(8))],
                        ins=[ar_in.opt()], outs=[ar_out.opt()])
                nc.sync.dma_start(hsum[:], ar_out[:])

            # ---------------- tanh, hq, bits2 ----------------
            bits2 = cp.tile([128, 4 * B], BF16, tag="bits2")
            hqbf = cp.tile([128, 4 * B], BF16, tag="hqbf")
            for fc in range(4):
                ht = wp.tile([128, B], F32, tag="w32")
                nc.scalar.activation(ht[:], hsum[:, fc * B:(fc + 1) * B], ACTF.Tanh,
                                     bias=0.0, scale=KAPPA)
                if debug:
                    nc.sync.dma_start(hdbg[:, fc * B:(fc + 1) * B], ht[:])
                hc = wp.tile([128, B], F32, tag="w32")
                nc.vector.tensor_scalar(out=hc[:], in0=ht[:], scalar1=0.0, scalar2=1.0,
                                        op0=OP.max, op1=OP.min)
                hq = wp.tile([128, B], I32, tag="wi32")
                nc.vector.tensor_scalar(out=hq[:], in0=hc[:], scalar1=255.0, scalar2=0.0,
                                        op0=OP.mult, op1=OP.add)
                nc.vector.tensor_scalar(out=hqbf[:, fc * B:(fc + 1) * B], in0=hq[:],
                                        scalar1=1.0, scalar2=0.0, op0=OP.mult, op1=OP.add)
                bsh = wp.tile([128, B], I32, tag="wi32")
                nc.vector.tensor_scalar(out=bsh[:], in0=hq[:], scalar1=sb_cc[:, 5:6],
                                        scalar2=-0.499, op0=OP.mult, op1=OP.add)
                half = wp.tile([128, B], I32, tag="wi32")
                nc.vector.tensor_scalar(out=half[:], in0=bsh[:], scalar1=0.5,
                                        scalar2=-0.499, op0=OP.mult, op1=OP.add)
                nc.vector.scalar_tensor_tensor(out=bits2[:, fc * B:(fc + 1) * B],
                                               in0=half[:], scalar=-2.0, in1=bsh[:],
                                               op0=OP.mult, op1=OP.add)

            # hqsum row
            pshq = psM.tile([1, B], F32, tag="m")
            for fc in range(4):
                for h in range(BH):
                    nc.tensor.matmul(pshq[:, h * NB:(h + 1) * NB], sb_o128[:],
                                     hqbf[:, fc * B + h * NB: fc * B + (h + 1) * NB],
                                     start=(fc == 0), stop=(fc == 3))
            hrow = rp.tile([1, B], F32, tag="rowB")
            nc.vector.tensor_scalar(out=hrow[:], in0=pshq[:],
                                    scalar1=float(np.float32(-1.0 / (255.0 * KAPPA * 8.0))),
                                    scalar2=0.0, op0=OP.mult, op1=OP.add)

            # ---------------- dummy D2 ----------------
            psD2 = psM.tile([S2, B], F32, tag="m")
            for s in range(S2):
                for h in range(BH):
                    nc.tensor.matmul(psD2[:, h * NB:(h + 1) * NB],
                                     sb_e42[:, s * S2:(s + 1) * S2],
                                     bits2[:, s * B + h * NB: s * B + (h + 1) * NB],
                                     start=(s == 0), stop=(s == S2 - 1))
            D2sb = wp.tile([S2, B], F32, tag="w32")
            nc.vector.tensor_scalar(out=D2sb[:], in0=psD2[:], scalar1=C03, scalar2=0.0,
                                    op0=OP.mult, op1=OP.add)
            mxD2 = tp.tile([S2, 1], F32, tag="t2")
            mnD2 = tp.tile([S2, 1], F32, tag="t2")
            nc.vector.tensor_reduce(out=mxD2[:], in_=D2sb[:], axis=AX.X, op=OP.max)
            nc.vector.tensor_reduce(out=mnD2[:], in_=D2sb[:], axis=AX.X, op=OP.min)
            dD2 = tp.tile([S2, 1], F32, tag="t2")
            nc.vector.tensor_tensor(out=dD2[:], in0=mxD2[:], in1=mnD2[:], op=OP.subtract)
            rcD2 = tp.tile([S2, 1], F32, tag="t2")
            nc.vector.reciprocal(rcD2[:], dD2[:])
            mkD2 = tp.tile([S2, 1], F32, tag="t2")
            nc.vector.tensor_scalar(out=mkD2[:], in0=dD2[:], scalar1=0.0, scalar2=0.0,
                                    op0=OP.is_gt, op1=OP.add)
            rD2 = tp.tile([S2, 1], F32, tag="t2")
            nc.vector.tensor_scalar(out=rD2[:], in0=rcD2[:], scalar1=mkD2[:, 0:1],
                                    scalar2=RSCALE, op0=OP.mult, op1=OP.mult)
            rD2n = tp.tile([S2, 1], F32, tag="t2")
            nc.vector.tensor_scalar(out=rD2n[:], in0=rD2[:], scalar1=-1.0, scalar2=0.0,
                                    op0=OP.mult, op1=OP.add)
            bD2 = tp.tile([S2, 1], F32, tag="t2")
            nc.vector.tensor_scalar(out=bD2[:], in0=mnD2[:], scalar1=rD2n[:, 0:1],
                                    scalar2=-0.5, op0=OP.mult, op1=OP.add)
            stD2 = tp.tile([S2, 1], F32, tag="t2")
            nc.vector.tensor_scalar(out=stD2[:], in0=dD2[:], scalar1=STEPS, scalar2=0.0,
                                    op0=OP.mult, op1=OP.add)
            idxD2 = wp.tile([S2, B], I32, tag="wi32")
            nc.vector.tensor_scalar(out=idxD2[:], in0=D2sb[:], scalar1=rD2[:, 0:1],
                                    scalar2=bD2[:, 0:1], op0=OP.mult, op1=OP.add)
            DqD2 = wp.tile([S2, B], F32, tag="w32")
            nc.vector.tensor_scalar(out=DqD2[:], in0=idxD2[:], scalar1=stD2[:, 0:1],
                                    scalar2=mnD2[:, 0:1], op0=OP.mult, op1=OP.add)

            # ---------------- layer-2 main ----------------
            def emit_p2(ps, s2):
                wof = s2 * KSL * F2
                for h in range(BH):
                    nc.tensor.matmul(ps[:, h * NB:(h + 1) * NB],
                                     sb_g2h[:, wof:wof + KSL * F2],
                                     bits2[:, s2 * B + h * NB: s2 * B + (h + 1) * NB],
                                     start=True, stop=False)
                for h in range(BH):
                    nc.tensor.matmul(ps[:, h * NB:(h + 1) * NB],
                                     sb_g2l[:, wof:wof + KSL * F2],
                                     bits2[:, s2 * B + h * NB: s2 * B + (h + 1) * NB],
                                     start=False, stop=True)

            M2 = KSL * F2
            maxC2 = lp.tile([M2, 2 * S2], F32, tag="c2")
            for s2 in range(S2):
                ps = psA.tile([M2, B], F32, tag="p1")
                emit_p2(ps, s2)
                nc.vector.tensor_reduce(out=maxC2[:, s2:s2 + 1], in_=ps[:],
                                        axis=AX.X, op=OP.max)
                nc.vector.tensor_reduce(out=maxC2[:, S2 + s2:S2 + s2 + 1], in_=ps[:],
                                        axis=AX.X, op=OP.min)
            pt2a = psM.tile([S2, M2], F32, tag="m")
            nc.tensor.transpose(pt2a[:], maxC2[:, 0:S2], sb_id[0:M2, 0:M2])
            tca = tp.tile([S2, M2], F32, tag="tcc")
            nc.vector.tensor_copy(tca[:], pt2a[:])
            pt2b = psM.tile([S2, M2], F32, tag="m")
            nc.tensor.transpose(pt2b[:], maxC2[:, S2:2 * S2], sb_id[0:M2, 0:M2])
            tcb = tp.tile([S2, M2], F32, tag="tcc")
            nc.vector.tensor_copy(tcb[:], pt2b[:])
            mx2 = tp.tile([S2, KSL], F32, tag="q")
            mn2 = tp.tile([S2, KSL], F32, tag="q")
            nc.vector.tensor_reduce(out=mx2[:],
                                    in_=tca[:].rearrange("p (k f) -> p k f", f=F2),
                                    axis=AX.X, op=OP.max)
            nc.vector.tensor_reduce(out=mn2[:],
                                    in_=tcb[:].rearrange("p (k f) -> p k f", f=F2),
                                    axis=AX.X, op=OP.min)
            d2 = tp.tile([S2, KSL], F32, tag="q")
            nc.vector.tensor_tensor(out=d2[:], in0=mx2[:], in1=mn2[:], op=OP.subtract)
            rc2 = tp.tile([S2, KSL], F32, tag="q")
            nc.vector.reciprocal(rc2[:], d2[:])
            mk2 = tp.tile([S2, KSL], F32, tag="q")
            nc.vector.tensor_scalar(out=mk2[:], in0=d2[:], scalar1=0.0, scalar2=0.0,
                                    op0=OP.is_gt, op1=OP.add)
            r2t = tp.tile([S2, KSL], F32, tag="q")
            nc.vector.tensor_tensor(out=r2t[:], in0=rc2[:], in1=mk2[:], op=OP.mult)
            nc.vector.tensor_scalar(out=r2t[:], in0=r2t[:], scalar1=RSCALE, scalar2=0.0,
                                    op0=OP.mult, op1=OP.add)
            b2t = tp.tile([S2, KSL], F32, tag="q")
            nc.vector.tensor_tensor(out=b2t[:], in0=mn2[:], in1=r2t[:], op=OP.mult)
            nc.vector.tensor_scalar(out=b2t[:], in0=b2t[:], scalar1=-1.0, scalar2=-0.5,
                                    op0=OP.mult, op1=OP.add)
            st2t = tp.tile([S2, KSL], F32, tag="q")
            nc.vector.tensor_scalar(out=st2t[:], in0=d2[:], scalar1=STEPS, scalar2=0.0,
                                    op0=OP.mult, op1=OP.add)
            c2t = tp.tile([S2, KSL], F32, tag="q")
            nc.vector.tensor_tensor(out=c2t[:], in0=st2t[:], in1=sb_s2t[:], op=OP.mult)
            smn2 = tp.tile([S2, KSL], F32, tag="q")
            nc.vector.tensor_tensor(out=smn2[:], in0=mn2[:], in1=sb_s2t[:], op=OP.mult)
            psg2 = psM.tile([1, KSL], F32, tag="m")
            nc.tensor.matmul(psg2[:], sb_o7[0:S2, :], smn2[:], start=True, stop=True)
            sg2r = tp.tile([1, KSL], F32, tag="sg")
            nc.vector.tensor_copy(sg2r[:], psg2[:])
            sg2 = tp.tile([1, 1], F32, tag="sg1")
            nc.vector.tensor_reduce(out=sg2[:], in_=sg2r[:], axis=AX.X, op=OP.add)
            # transpose r2/b2/c2 ([s2,k] -> [k,s2]) via PE, then per-s2 bcast
            psT = psM.tile([S2, 3 * KSL], F32, tag="m")
            nc.tensor.transpose(psT[:, 0:KSL], r2t[:], sb_id[0:S2, 0:S2])
            nc.tensor.transpose(psT[:, KSL:2 * KSL], b2t[:], sb_id[0:S2, 0:S2])
            nc.tensor.transpose(psT[:, 2 * KSL:3 * KSL], c2t[:], sb_id[0:S2, 0:S2])
            sT = tp.tile([S2, 3 * KSL], F32, tag="sT")
            nc.vector.tensor_copy(sT[:], psT[:])
            scl2 = []
            for s2 in range(S2):
                rhs = tp.tile([KSL, 3], F32, tag="rh")
                nc.vector.tensor_copy(rhs[:, 0:1], sT[:, s2:s2 + 1])
                nc.vector.tensor_copy(rhs[:, 1:2], sT[:, KSL + s2:KSL + s2 + 1])
                nc.vector.tensor_copy(rhs[:, 2:3], sT[:, 2 * KSL + s2:2 * KSL + s2 + 1])
                psc = psM.tile([M2, 3], F32, tag="m")
                nc.tensor.matmul(psc[:], sb_bind[:], rhs[:], start=True, stop=True)
                sc = tp.tile([M2, 3], F32, tag="sc%d" % s2)
                nc.vector.tensor_copy(sc[:], psc[:])
                scl2.append(sc)
            acc2 = cp.tile([M2, B], F32, tag="acc2")
            for s2 in range(S2):
                ps = psB.tile([M2, B], F32, tag="p2")
                emit_p2(ps, s2)
                idxt = ip.tile([M2, B], I32, tag="ix")
                nc.scalar.activation(idxt[:], ps[:], ACTF.Identity,
                                     bias=scl2[s2][:, 1:2], scale=scl2[s2][:, 0:1])
                if s2 == 0:
                    nc.vector.tensor_scalar(out=acc2[:], in0=idxt[:],
                                            scalar1=scl2[s2][:, 2:3], scalar2=0.0,
                                            op0=OP.mult, op1=OP.add)
                else:
                    nc.vector.scalar_tensor_tensor(out=acc2[:], in0=idxt[:],
                                                   scalar=scl2[s2][:, 2:3], in1=acc2[:],
                                                   op0=OP.mult, op1=OP.add)
            # dummy2 row + sigma2 + hq row -> late2row
            psd2 = psM.tile([1, B], F32, tag="m")
            for h in range(BH):
                nc.tensor.matmul(psd2[:, h * NB:(h + 1) * NB], sb_o7[0:S2, :],
                                 DqD2[:, h * NB:(h + 1) * NB], start=True, stop=True)
            l2a = rp.tile([1, B], F32, tag="rowB")
            nc.vector.tensor_scalar(out=l2a[:], in0=psd2[:], scalar1=sb_cc[0:1, 4:5],
                                    scalar2=sg2[:, 0:1], op0=OP.mult, op1=OP.add)
            l2b = rp.tile([1, B], F32, tag="rowB")
            nc.vector.tensor_tensor(out=l2b[:], in0=l2a[:], in1=hrow[:], op=OP.add)
            # fold 40 -> 10 plus rank-1 late row
            psf = psM.tile([F2, B], F32, tag="m")
            for h in range(BH):
                nc.tensor.matmul(psf[:, h * NB:(h + 1) * NB], sb_f40[:],
                                 acc2[:, h * NB:(h + 1) * NB], start=True, stop=False)
            for h in range(BH):
                nc.tensor.matmul(psf[:, h * NB:(h + 1) * NB], sb_orow[0:1, 0:F2],
                                 l2b[:, h * NB:(h + 1) * NB], start=False, stop=True)
            o2a = rp.tile([F2, B], F32, tag="rowB")
            nc.vector.tensor_copy(o2a[:], psf[:])
            ar2i = dp.tile([F2, B], F32, tag="ar2i")
            ar2o = dp.tile([F2, B], F32, tag="ar2o")
            nc.sync.dma_start(ar2i[:], o2a[:])
            if abl_noar:
                nc.sync.dma_start(ar2o[:], ar2i[:])
            else:
                nc.gpsimd.collective_compute(
                    "AllReduce", OP.add, replica_groups=[list(range(8))],
                    ins=[ar2i.opt()], outs=[ar2o.opt()])
            o2b = rp.tile([F2, B], F32, tag="rowB")
            nc.sync.dma_start(o2b[:], ar2o[:])
            o2c = rp.tile([F2, B], F32, tag="rowB")
            nc.vector.tensor_scalar(out=o2c[:], in0=o2b[:], scalar1=KAPPA, scalar2=0.0,
                                    op0=OP.mult, op1=OP.add)
            nc.sync.dma_start(out2d[:], o2c[:])

    nc.compile()
    _NC_CACHE[key] = nc
    return nc


# ------------------------------------------------------------------ runner
class Runner:
    """Cached jit over the bass_exec custom call with device-resident inputs.

    Same lowering path bass_utils.run_bass_kernel_spmd takes under axon
    (bass2jax.run_bass_via_pjrt), but the jitted callable and the
    device-placed input buffers persist across calls, so a warm call ships
    no input data over the axon tunnel.
    """

    def __init__(self, nc):
        install_neuronx_cc_hook()
        self.nc = nc
        partition_name = (nc.partition_id_tensor.name
                          if nc.partition_id_tensor else None)
        in_names, out_names, out_avals, zero_shapes = [], [], [], []
        for alloc in nc.m.functions[0].allocations:
            if not isinstance(alloc, mybir.MemoryLocationSet):
                continue
            name = alloc.memorylocations[0].name
            if alloc.kind == "ExternalInput":
                if name != partition_name:
                    in_names.append(name)
            elif alloc.kind == "ExternalOutput":
                out_names.append(name)
                shape = tuple(alloc.tensor_shape)
                dtype = mybir.dt.np(alloc.dtype)
                out_avals.append(jax.core.ShapedArray(shape, dtype))
                zero_shapes.append((shape, dtype))
        self.in_names = in_names
        self.out_names = out_names
        n_params = len(in_names)
        all_names = list(in_names) + list(out_names)
        if partition_name is not None:
            all_names.append(partition_name)

        def _body(*args):
            operands = list(args)
            if partition_name is not None:
                operands.append(partition_id_tensor())
            return tuple(_bass_exec_p.bind(
                *operands,
                out_avals=tuple(out_avals),
                in_names=tuple(all_names),
                out_names=tuple(out_names),
                lowering_input_output_aliases=(),
                sim_require_finite=True,
                sim_require_nnan=True,
                nc=nc,
            ))

        devices = jax.devices()[:N_CORES]
        self.mesh = Mesh(np.asarray(devices), ("core",))
        n_outs = len(out_names)
        in_specs = (PartitionSpec("core"),) * (n_params + n_outs)
        out_specs = (PartitionSpec("core"),) * n_outs
        self.fn = jax.jit(shard_map(_body, mesh=self.mesh, in_specs=in_specs,
                                    out_specs=out_specs, check_rep=False))
        self.sharding = NamedSharding(self.mesh, PartitionSpec("core"))
        self.dev_zeros = [
            jax.device_put(np.zeros((N_CORES * s[0], *s[1:]), dt), self.sharding)
            for s, dt in zero_shapes]
        self.dev_in = None

    def put_inputs(self, in_maps):
        """Concat per-core input maps and place on devices (sharded by core)."""
        self.dev_in = [
            jax.device_put(
                np.concatenate([np.asarray(m[name]) for m in in_maps], axis=0),
                self.sharding)
            for name in self.in_names]
        jax.block_until_ready(self.dev_in)

    def execute(self):
        """One dispatch (async); returns jax output arrays, concat by core."""
        return self.fn(*self.dev_in, *self.dev_zeros)

    def run(self):
        outs = self.execute()
        jax.block_until_ready(outs)
        return {name: np.asarray(outs[i]) for i, name in enumerate(self.out_names)}


def get_runner(B, nrep=1, debug=False, store_p=0, gp_accum=False,
               abl_nop2mm=False, abl_noar=False, gp_minred=False, gp_max=False,
             ar_chunk=False):
    key = (B, nrep, debug, store_p, gp_accum, abl_nop2mm, abl_noar, gp_minred,
           gp_max, ar_chunk)
    if key not in _RUNNER_CACHE:
        _RUNNER_CACHE[key] = Runner(build_nc(
            B, nrep=nrep, debug=debug, store_p=store_p, gp_accum=gp_accum,
            abl_nop2mm=abl_nop2mm, abl_noar=abl_noar, gp_minred=gp_minred))
    return _RUNNER_CACHE[key]


# ------------------------------------------------------------------ driver
def run_cores(inputs, B=1024, want_debug=False, nrep=1):
    shared, per_core = host_prepare(inputs["x"], inputs["w1"], inputs["w3"],
                                    inputs["noise1"], inputs["noise3"], B)
    runner = get_runner(B, nrep=nrep, debug=want_debug, gp_max=True)
    runner.put_inputs([{**shared, **pc} for pc in per_core])
    res = runner.run()
    out = res["out2"][0:F2].T.astype(np.float32)  # core 0 slice, [B, F2]
    if want_debug:
        return out, res
    return out


def kernel(**inputs):
    return run_cores(inputs, B=1024)


# revision 22
# speedup vs baseline: 4051.2539x; 1.0439x over previous
"""Trainium2 Bass kernel for nn_DFANet (analog PIM crossbar MLP emulation).

Sharding: input-bit-plane parallel — core c owns input bit i=c for layer 1
and hq bit i2=c for layer 2. All ADC min/max groups are then core-local;
one fp32 sum-AllReduce of the accumulator happens at each layer boundary.

Self-contained: hardcodes all shapes; host precomputes bit-planes and
conductance tensors (exact fp32 mirror of the reference formulas, split
into bf16 hi+lo pairs so PE products with 0/1 bits are fp32-grade).

Execution path: a cached jax.jit(shard_map) over the bass_exec custom
call (the same machinery bass_utils.run_bass_kernel_spmd uses under
axon), with all input tensors device-resident so repeated calls ship no
data. build_nc(nrep=N) unrolls the whole program N times inside one NEFF
for steady-state per-iteration timing.
"""
import math
import sys

import numpy as np

sys.path.insert(0, "/opt/trn_rl_repo")

import ml_dtypes  # noqa: E402
import jax  # noqa: E402
from jax.sharding import Mesh, PartitionSpec, NamedSharding  # noqa: E402
from jax.experimental.shard_map import shard_map  # noqa: E402

import concourse.bass as bass  # noqa: E402
import concourse.mybir as mybir  # noqa: E402
import concourse.tile as tile  # noqa: E402
import concourse.bacc as bacc  # noqa: E402
from concourse import bass_utils  # noqa: E402
from concourse.bass2jax import (  # noqa: E402
    _bass_exec_p,
    partition_id_tensor,
    install_neuronx_cc_hook,
)

F32 = mybir.dt.float32
BF16 = mybir.dt.bfloat16
I32 = mybir.dt.int32
AX = mybir.AxisListType
OP = mybir.AluOpType
ACTF = mybir.ActivationFunctionType

# problem constants
I_BITS = 8
S1, S2 = 7, 4
KSL = 4            # weight slices
F1, F2 = 512, 10
N1, N2 = 784, 512
NP1 = S1 * 128
CR = 4.0
LOWER, UPPER = np.float32(1.0 / 10.0), np.float32(1.0)
GLO = np.float32(np.float32(CR - 1.0) * LOWER)      # (cr-1)*lower
GSC = np.float32(UPPER - LOWER)                     # 0.9
KAPPA = float(np.float32(2.0 / (0.9 * 255.0 * 255.0)))
RSCALE = float(np.float32(32.0 * (1.0 - 2.0 ** -22)))
STEPS = float(np.float32(2.0 ** -5))
C03 = float(np.float32(3.0) * np.float32(0.1))      # (cr-1)*lower as f32

N_CORES = 8

_NC_CACHE = {}
_RUNNER_CACHE = {}


# ----------------------------------------------------------------- host prep
def _qweights(w):
    """Xi -> slices -> conductances, mirroring reference fp32 ops exactly."""
    w = np.asarray(w, np.float32)
    Xi = np.clip(np.round((w + np.float32(1.0)) * np.float32(0.5) * np.float32(255.0)),
                 0.0, 255.0).astype(np.float32)
    return Xi


def _gtensor(Xi, noise, S):
    """g[f, s*128+a, k] fp32, padded to S*128 rows; mirrors reference."""
    F, N = Xi.shape
    Np = S * 128
    Xi = np.pad(Xi, ((0, 0), (0, Np - N)))
    kpow = (np.float32(CR) ** np.arange(KSL)).astype(np.float32)
    slc = np.mod(np.floor(Xi[..., None] / kpow), np.float32(CR)).astype(np.float32)
    g = slc * GSC + GLO
    g = (g * (np.float32(1.0) + np.float32(0.05) * np.asarray(noise, np.float32))).astype(np.float32)
    return g  # [F, Np, K]


def _hi_lo(x):
    hi = x.astype(ml_dtypes.bfloat16)
    lo = (x - hi.astype(np.float32)).astype(ml_dtypes.bfloat16)
    return hi, lo


def host_prepare(x, w1, w3, noise1, noise3, B):
    """Returns (shared dict, per-core list of dicts) of DRAM input arrays."""
    x = np.asarray(x, np.float32)[:B]
    xq = np.round(np.clip(x, 0.0, 1.0) * np.float32(255.0)).astype(np.float32)  # [B, N1]
    xq_pad = np.pad(xq, ((0, 0), (0, NP1 - N1)))
    zpow = (np.float32(2.0) ** np.arange(I_BITS)).astype(np.float32)
    bits = np.mod(np.floor(xq_pad[..., None] / zpow), np.float32(2.0))  # [B, NP1, I]
    # bitsT per i: [128, S1*B] bf16, block s cols = bits[:, s*128+a, i].T
    bitsT = np.transpose(bits, (2, 1, 0))  # [I, NP1, B]
    bitsT = bitsT.reshape(I_BITS, S1, 128, B)

    g1 = _gtensor(_qweights(w1), noise1, S1)          # [512, 896, 4]
    # lhsT layout per (k,s): [a=128, f=512]; slot sk = k*7+s
    g1l = np.transpose(g1.reshape(F1, S1, 128, KSL), (3, 1, 2, 0))  # [K, S1, 128, F1]
    g1flat = g1l.reshape(KSL * S1, 128, F1).transpose(1, 0, 2).reshape(128, KSL * S1 * F1)
    g1hi, g1lo = _hi_lo(np.ascontiguousarray(g1flat))

    g2 = _gtensor(_qweights(w3), noise3, S2)          # [10, 512, 4]
    # per s2: [a=128, 40] with col k*10+f
    g2l = np.transpose(g2.reshape(F2, S2, 128, KSL), (1, 2, 3, 0))  # [S2, 128, K, F2]
    g2flat = g2l.reshape(S2, 128, KSL * F2).transpose(1, 0, 2).reshape(128, S2 * KSL * F2)
    g2hi, g2lo = _hi_lo(np.ascontiguousarray(g2flat))

    xqsum = xq.sum(axis=1, dtype=np.float32).astype(np.float32)   # [B]
    row1 = (-(xqsum / np.float32(255.0)) / np.float32(KAPPA) / np.float32(8.0)
            ).astype(np.float32)[None, :]  # [1, B]

    e7 = np.zeros((128, S1 * S1), np.float32)
    for s in range(S1):
        e7[:, s * S1 + s] = 1.0
    e42 = np.zeros((128, S2 * S2), np.float32)
    for s in range(S2):
        e42[:, s * S2 + s] = 1.0
    ones128 = np.ones((128, 1), np.float32)
    onesrow = np.ones((1, 128), np.float32)
    ones7 = np.ones((S1, 1), np.float32)
    fold40 = np.zeros((KSL * F2, F2), np.float32)
    for k in range(KSL):
        for j in range(F2):
            fold40[k * F2 + j, j] = 1.0
    blockind = np.zeros((KSL, KSL * F2), np.float32)
    for k in range(KSL):
        blockind[k, k * F2:(k + 1) * F2] = 1.0
    ident = np.eye(128, dtype=np.float32)
    scal2t = np.zeros((S2, KSL), np.float32)

    shared = dict(
        g1hi=np.asarray(g1hi), g1lo=np.asarray(g1lo),
        g2hi=np.asarray(g2hi), g2lo=np.asarray(g2lo),
        e7=e7.astype(ml_dtypes.bfloat16), e42=e42.astype(ml_dtypes.bfloat16),
        ones128=ones128.astype(ml_dtypes.bfloat16),
        onesrow=onesrow, ones7=ones7, fold40=fold40, blockind=blockind,
        ident=ident, row1=row1,
    )
    per_core = []
    for c in range(8):
        sc = np.float32(2.0 ** c)
        cconst = np.zeros((128, 16), np.float32)
        for k in range(KSL):
            cconst[:, k] = sc * np.float32(4.0 ** k)
        cconst[:, 4] = -np.float32(85.0) * sc
        cconst[:, 5] = np.float32(2.0 ** -c)
        st2 = scal2t.copy()
        for k in range(KSL):
            st2[:, k] = sc * np.float32(4.0 ** k)
        btc = np.ascontiguousarray(
            bitsT[c].transpose(1, 0, 2).reshape(128, S1 * B)).astype(ml_dtypes.bfloat16)
        per_core.append(dict(bitsT=np.asarray(btc), cconst=cconst, scal2t=st2))
    return shared, per_core


# ------------------------------------------------------------- bass program
def build_nc(B, nrep=1, debug=False, store_p=0, gp_accum=False,
             abl_nop2mm=False, abl_noar=False, gp_minred=False, gp_max=False,
             ar_chunk=False, mm1024=False):
    key = (B, nrep, debug, store_p, gp_accum, abl_nop2mm, abl_noar, gp_minred,
           gp_max, ar_chunk, mm1024)
    if key in _NC_CACHE:
        return _NC_CACHE[key]
    BH = B // 512 if B >= 512 else 1
    NB = min(B, 512)                      # matmul moving chunk
    nc = bacc.Bacc("TRN2", target_bir_lowering=False, debug=False,
                   num_devices=8)

    def din(name, shape, dt):
        return nc.dram_tensor(name, list(shape), dt, kind="ExternalInput")[:]

    bitsT = din("bitsT", (128, S1 * B), BF16)
    g1hi = din("g1hi", (128, KSL * S1 * F1), BF16)
    g1lo = din("g1lo", (128, KSL * S1 * F1), BF16)
    g2hi = din("g2hi", (128, S2 * KSL * F2), BF16)
    g2lo = din("g2lo", (128, S2 * KSL * F2), BF16)
    e7 = din("e7", (128, S1 * S1), BF16)
    e42 = din("e42", (128, S2 * S2), BF16)
    ones128 = din("ones128", (128, 1), BF16)
    onesrow = din("onesrow", (1, 128), F32)
    ones7 = din("ones7", (S1, 1), F32)
    fold40 = din("fold40", (KSL * F2, F2), F32)
    blockind = din("blockind", (KSL, KSL * F2), F32)
    ident = din("ident", (128, 128), F32)
    row1 = din("row1", (1, B), F32)
    cconst = din("cconst", (128, 16), F32)
    scal2t = din("scal2t", (S2, KSL), F32)

    out2d = nc.dram_tensor("out2", [F2, B], F32, kind="ExternalOutput")[:]
    hdbg = (nc.dram_tensor("h_dbg", [128, 4 * B], F32, kind="ExternalOutput")[:]
            if debug else None)

    with tile.TileContext(nc) as tc:
        with (
            tc.tile_pool(name="const", bufs=1) as cp,
            tc.tile_pool(name="work", bufs=4) as wp,
            tc.tile_pool(name="idx", bufs=3) as ip,
            tc.tile_pool(name="tiny", bufs=16) as tp,
            tc.tile_pool(name="coll", bufs=2) as lp,
            tc.tile_pool(name="rows", bufs=3) as rp,
            tc.tile_pool(name="pstore", bufs=1) as pp,
            tc.tile_pool(name="ps_a", bufs=2, space="PSUM") as psA,
            tc.tile_pool(name="ps_b", bufs=1, space="PSUM") as psB,
            tc.tile_pool(name="ps_m", bufs=1, space="PSUM") as psM,
            tc.tile_pool(name="dram", bufs=1, space="DRAM") as dp,
        ):
          for _rep in range(nrep):
            _par = _rep % 2
            # ---- load constants to SBUF
            def load(ap, shape, dt, tag):
                t = cp.tile(list(shape), dt, tag=tag)
                nc.sync.dma_start(t[:], ap)
                return t

            sb_bits = load(bitsT, (128, S1 * B), BF16, tag="sb_bits")
            sb_g1h = load(g1hi, (128, KSL * S1 * F1), BF16, tag="sb_g1h")
            sb_g1l = load(g1lo, (128, KSL * S1 * F1), BF16, tag="sb_g1l")
            sb_g2h = load(g2hi, (128, S2 * KSL * F2), BF16, tag="sb_g2h")
            sb_g2l = load(g2lo, (128, S2 * KSL * F2), BF16, tag="sb_g2l")
            sb_e7 = load(e7, (128, S1 * S1), BF16, tag="sb_e7")
            sb_e42 = load(e42, (128, S2 * S2), BF16, tag="sb_e42")
            sb_o128 = load(ones128, (128, 1), BF16, tag="sb_o128")
            sb_orow = load(onesrow, (1, 128), F32, tag="sb_orow")
            sb_o7 = load(ones7, (S1, 1), F32, tag="sb_o7")
            sb_f40 = load(fold40, (KSL * F2, F2), F32, tag="sb_f40")
            sb_bind = load(blockind, (KSL, KSL * F2), F32, tag="sb_bind")
            sb_id = load(ident, (128, 128), F32, tag="sb_id")
            sb_row1 = load(row1, (1, B), F32, tag="sb_row1")
            sb_cc = load(cconst, (128, 16), F32, tag="sb_cc")
            sb_s2t = load(scal2t, (S2, KSL), F32, tag="sb_s2t")

            acc1 = cp.tile([128, 4 * B], F32, tag="acc1")
            sigc = cp.tile([S1, KSL], F32, tag="sigc")

            # PE warm-up on every DMA-loaded constant it will read later, so
            # later matmuls don't need a third (DMA) sync-wait slot.
            warm = psM.tile([1, 16], F32, tag="m")
            for j, t in enumerate([sb_id, sb_bind, sb_f40, sb_orow, sb_o7,
                                   sb_g1h, sb_g1l, sb_g2h, sb_g2l, sb_e7,
                                   sb_e42, sb_o128, sb_bits, sb_row1]):
                nc.tensor.matmul(warm[0:1, j:j + 1], t[0:1, 0:1], t[0:1, 0:1],
                                 start=True, stop=True)

            def emit_p1(ps, wslice_hi, wslice_lo, rhs_base):
                if mm1024:
                    nc.tensor.matmul(ps[:, 0:B], wslice_hi,
                                     sb_bits[:, rhs_base: rhs_base + B],
                                     start=True, stop=False)
                    nc.tensor.matmul(ps[:, 0:B], wslice_lo,
                                     sb_bits[:, rhs_base: rhs_base + B],
                                     start=False, stop=True)
                    return
                for h in range(BH):
                    nc.tensor.matmul(ps[:, h * NB:(h + 1) * NB], wslice_hi,
                                     sb_bits[:, rhs_base + h * NB: rhs_base + (h + 1) * NB],
                                     start=True, stop=False)
                for h in range(BH):
                    nc.tensor.matmul(ps[:, h * NB:(h + 1) * NB], wslice_lo,
                                     sb_bits[:, rhs_base + h * NB: rhs_base + (h + 1) * NB],
                                     start=False, stop=True)

            # ---------------- dummy D1 ----------------
            psD = psM.tile([S1, B], F32, tag="m")
            for s in range(S1):
                for h in range(BH):
                    nc.tensor.matmul(psD[:, h * NB:(h + 1) * NB],
                                     sb_e7[:, s * S1:(s + 1) * S1],
                                     sb_bits[:, s * B + h * NB: s * B + (h + 1) * NB],
                                     start=(s == 0), stop=(s == S1 - 1))
            Dsb = wp.tile([S1, B], F32, tag="w32")
            nc.vector.tensor_scalar(out=Dsb[:], in0=psD[:], scalar1=C03, scalar2=0.0,
                                    op0=OP.mult, op1=OP.add)
            mxD = tp.tile([S1, 1], F32, tag="t")
            mnD = tp.tile([S1, 1], F32, tag="t")
            nc.vector.tensor_reduce(out=mxD[:], in_=Dsb[:], axis=AX.X, op=OP.max)
            nc.vector.tensor_reduce(out=mnD[:], in_=Dsb[:], axis=AX.X, op=OP.min)
            dD = tp.tile([S1, 1], F32, tag="t")
            nc.vector.tensor_tensor(out=dD[:], in0=mxD[:], in1=mnD[:], op=OP.subtract)
            rcD = tp.tile([S1, 1], F32, tag="t")
            nc.vector.reciprocal(rcD[:], dD[:])
            mkD = tp.tile([S1, 1], F32, tag="t")
            nc.vector.tensor_scalar(out=mkD[:], in0=dD[:], scalar1=0.0, scalar2=0.0,
                                    op0=OP.is_gt, op1=OP.add)
            rD = tp.tile([S1, 1], F32, tag="t")
            nc.vector.tensor_scalar(out=rD[:], in0=rcD[:], scalar1=mkD[:, 0:1],
                                    scalar2=RSCALE, op0=OP.mult, op1=OP.mult)
            rDn = tp.tile([S1, 1], F32, tag="t")
            nc.vector.tensor_scalar(out=rDn[:], in0=rD[:], scalar1=-1.0, scalar2=0.0,
                                    op0=OP.mult, op1=OP.add)
            bD = tp.tile([S1, 1], F32, tag="t")
            nc.vector.tensor_scalar(out=bD[:], in0=mnD[:], scalar1=rDn[:, 0:1],
                                    scalar2=-0.5, op0=OP.mult, op1=OP.add)
            stD = tp.tile([S1, 1], F32, tag="t")
            nc.vector.tensor_scalar(out=stD[:], in0=dD[:], scalar1=STEPS, scalar2=0.0,
                                    op0=OP.mult, op1=OP.add)
            idxD = wp.tile([S1, B], I32, tag="wi32")
            nc.vector.tensor_scalar(out=idxD[:], in0=Dsb[:], scalar1=rD[:, 0:1],
                                    scalar2=bD[:, 0:1], op0=OP.mult, op1=OP.add)
            DqD = wp.tile([S1, B], F32, tag="w32")
            nc.vector.tensor_scalar(out=DqD[:], in0=idxD[:], scalar1=stD[:, 0:1],
                                    scalar2=mnD[:, 0:1], op0=OP.mult, op1=OP.add)

            # ---------------- layer-1 main loop ----------------
            for k in range(KSL):
                maxC = lp.tile([128, S1 * KSL], F32, tag="mx")
                minC = lp.tile([128, S1 * KSL], F32, tag="mn")
                gmx = lp.tile([1, S1 * KSL], F32, tag="gmx")
                pstore = {}
                # pass 1
                for s in range(S1):
                    for fc in range(4):
                        ps = psA.tile([128, B], F32, tag="p1")
                        wof = (k * S1 + s) * F1 + fc * 128
                        emit_p1(ps, sb_g1h[:, wof:wof + 128], sb_g1l[:, wof:wof + 128],
                                s * B)
                        if fc < store_p:
                            sbP = pp.tile([128, B], F32, tag="P%d_%d" % (s, fc))
                            nc.scalar.activation(sbP[:], ps[:], ACTF.Identity,
                                                 bias=0.0, scale=1.0)
                            pstore[(s, fc)] = sbP
                        if gp_max:
                            nc.gpsimd.tensor_reduce(
                                out=gmx[0:1, s * 4 + fc:s * 4 + fc + 1],
                                in_=ps[:], axis=AX.XYZWC, op=OP.max)
                        else:
                            nc.vector.tensor_reduce(out=maxC[:, s * 4 + fc:s * 4 + fc + 1],
                                                    in_=ps[:], axis=AX.X, op=OP.max)
                        nc.vector.tensor_reduce(out=minC[:, s * 4 + fc:s * 4 + fc + 1],
                                                in_=ps[:], axis=AX.X, op=OP.min)
                # combine k: fc-fold then transpose then partition fold
                red = tp.tile([128, 2 * S1], F32, tag="red")
                if not gp_max:
                    nc.vector.tensor_reduce(out=red[:, 0:S1],
                                            in_=maxC[:].rearrange("p (s f) -> p s f", f=4),
                                            axis=AX.X, op=OP.max)
                nc.vector.tensor_reduce(out=red[:, S1:2 * S1],
                                        in_=minC[:].rearrange("p (s f) -> p s f", f=4),
                                        axis=AX.X, op=OP.min)
                mx = tp.tile([S1, 1], F32, tag="t")
                if gp_max:
                    redr = tp.tile([1, S1], F32, tag="redr")
                    nc.vector.tensor_reduce(
                        out=redr[:], in_=gmx[:].rearrange("p (s f) -> p s f", f=4),
                        axis=AX.X, op=OP.max)
                    pmx = psM.tile([S1, 1], F32, tag="m")
                    nc.tensor.transpose(pmx[:], redr[:], sb_id[0:1, 0:1])
                    nc.vector.tensor_copy(mx[:], pmx[:])
                else:
                    ptm = psM.tile([S1, 128], F32, tag="m")
                    nc.tensor.transpose(ptm[:], red[:, 0:S1], sb_id[:])
                    tcm = tp.tile([S1, 128], F32, tag="tc")
                    nc.vector.tensor_copy(tcm[:], ptm[:])
                    nc.vector.tensor_reduce(out=mx[:], in_=tcm[:], axis=AX.X, op=OP.max)
                ptn = psM.tile([S1, 128], F32, tag="m")
                nc.tensor.transpose(ptn[:], red[:, S1:2 * S1], sb_id[:])
                tcn = tp.tile([S1, 128], F32, tag="tc")
                nc.vector.tensor_copy(tcn[:], ptn[:])
                mn = tp.tile([S1, 1], F32, tag="t")
                nc.vector.tensor_reduce(out=mn[:], in_=tcn[:], axis=AX.X, op=OP.min)
                d = tp.tile([S1, 1], F32, tag="t")
                nc.vector.tensor_tensor(out=d[:], in0=mx[:], in1=mn[:], op=OP.subtract)
                rc = tp.tile([S1, 1], F32, tag="t")
                nc.vector.reciprocal(rc[:], d[:])
                mk = tp.tile([S1, 1], F32, tag="t")
                nc.vector.tensor_scalar(out=mk[:], in0=d[:], scalar1=0.0, scalar2=0.0,
                                        op0=OP.is_gt, op1=OP.add)
                rr = tp.tile([S1, 1], F32, tag="t")
                nc.vector.tensor_scalar(out=rr[:], in0=rc[:], scalar1=mk[:, 0:1],
                                        scalar2=RSCALE, op0=OP.mult, op1=OP.mult)
                rrn = tp.tile([S1, 1], F32, tag="t")
                nc.vector.tensor_scalar(out=rrn[:], in0=rr[:], scalar1=-1.0, scalar2=0.0,
                                        op0=OP.mult, op1=OP.add)
                bb = tp.tile([S1, 1], F32, tag="t")
                nc.vector.tensor_scalar(out=bb[:], in0=mn[:], scalar1=rrn[:, 0:1],
                                        scalar2=-0.5, op0=OP.mult, op1=OP.add)
                stp = tp.tile([S1, 1], F32, tag="t")
                nc.vector.tensor_scalar(out=stp[:], in0=d[:], scalar1=STEPS, scalar2=0.0,
                                        op0=OP.mult, op1=OP.add)
                cc = tp.tile([S1, 1], F32, tag="t")
                nc.vector.tensor_scalar(out=cc[:], in0=stp[:], scalar1=sb_cc[0:S1, k:k + 1],
                                        scalar2=0.0, op0=OP.mult, op1=OP.add)
                nc.vector.tensor_scalar(out=sigc[:, k:k + 1], in0=mn[:],
                                        scalar1=sb_cc[0:S1, k:k + 1], scalar2=0.0,
                                        op0=OP.mult, op1=OP.add)
                # broadcast r/b/c to [128, 3*S1]: transpose cols to one row, rank-1
                prow = psM.tile([1, 3 * S1], F32, tag="m")
                nc.tensor.transpose(prow[:, 0:S1], rr[:], sb_id[0:S1, 0:S1])
                nc.tensor.transpose(prow[:, S1:2 * S1], bb[:], sb_id[0:S1, 0:S1])
                nc.tensor.transpose(prow[:, 2 * S1:3 * S1], cc[:], sb_id[0:S1, 0:S1])
                row21 = tp.tile([1, 3 * S1], F32, tag="r21")
                nc.vector.tensor_copy(row21[:], prow[:])
                pbc = psM.tile([128, 3 * S1], F32, tag="m")
                nc.tensor.matmul(pbc[:], sb_orow[:], row21[:], start=True, stop=True)
                bck = tp.tile([128, 3 * S1], F32, tag="bck")
                nc.vector.tensor_copy(bck[:], pbc[:])
                # pass 2
                acc_eng = nc.gpsimd if gp_accum else nc.vector
                prev_ps = [None]
                for s in range(S1):
                    for fc in range(4):
                        if (s, fc) in pstore:
                            psrc = pstore[(s, fc)]
                        elif abl_nop2mm and prev_ps[0] is not None:
                            psrc = prev_ps[0]   # timing ablation only: wrong data
                        else:
                            ps = psB.tile([128, B], F32, tag="p2")
                            wof = (k * S1 + s) * F1 + fc * 128
                            emit_p1(ps, sb_g1h[:, wof:wof + 128],
                                    sb_g1l[:, wof:wof + 128], s * B)
                            psrc = ps
                            prev_ps[0] = ps
                        idxt = ip.tile([128, B], I32, tag="ix")
                        nc.scalar.activation(idxt[:], psrc[:], ACTF.Identity,
                                             bias=bck[:, S1 + s:S1 + s + 1],
                                             scale=bck[:, s:s + 1])
                        asl = acc1[:, fc * B:(fc + 1) * B]
                        if k == 0 and s == 0:
                            acc_eng.tensor_scalar(out=asl, in0=idxt[:],
                                                  scalar1=bck[:, 2 * S1 + s:2 * S1 + s + 1],
                                                  scalar2=0.0, op0=OP.mult, op1=OP.add)
                        else:
                            acc_eng.scalar_tensor_tensor(
                                out=asl, in0=idxt[:],
                                scalar=bck[:, 2 * S1 + s:2 * S1 + s + 1],
                                in1=asl, op0=OP.mult, op1=OP.add)

            # ---------------- layer-1 tail: sigma, dummy, row1 ----------------
            psg = psM.tile([1, KSL], F32, tag="m")
            nc.tensor.matmul(psg[:], sb_o7[:], sigc[:], start=True, stop=True)
            sgr = tp.tile([1, KSL], F32, tag="sg")
            nc.vector.tensor_copy(sgr[:], psg[:])
            sg = tp.tile([1, 1], F32, tag="sg1")
            nc.vector.tensor_reduce(out=sg[:], in_=sgr[:], axis=AX.X, op=OP.add)
            psdr = psM.tile([1, B], F32, tag="m")
            for h in range(BH):
                nc.tensor.matmul(psdr[:, h * NB:(h + 1) * NB], sb_o7[:],
                                 DqD[:, h * NB:(h + 1) * NB], start=True, stop=True)
            late = rp.tile([1, B], F32, tag="rowB")
            nc.vector.tensor_scalar(out=late[:], in0=psdr[:], scalar1=sb_cc[0:1, 4:5],
                                    scalar2=sg[:, 0:1], op0=OP.mult, op1=OP.add)
            late2 = rp.tile([1, B], F32, tag="rowB")
            nc.vector.tensor_tensor(out=late2[:], in0=late[:], in1=sb_row1[:], op=OP.add)
            plate = psM.tile([128, B], F32, tag="m")
            for h in range(BH):
                nc.tensor.matmul(plate[:, h * NB:(h + 1) * NB], sb_orow[:],
                                 late2[:, h * NB:(h + 1) * NB], start=True, stop=True)
            hsum = cp.tile([128, 4 * B], F32, tag="hsum")
            if ar_chunk:
                # per-fc boundary pipeline: accumulate tail, AR chunk, fetch
                for fc in range(4):
                    asl = acc1[:, fc * B:(fc + 1) * B]
                    nc.vector.scalar_tensor_tensor(out=asl, in0=plate[:], scalar=1.0,
                                                   in1=asl, op0=OP.mult, op1=OP.add)
                    ari = dp.tile([128, B], F32, tag="ar_in%d_%d" % (fc, _par))
                    aro = dp.tile([128, B], F32, tag="ar_out%d_%d" % (fc, _par))
                    nc.sync.dma_start(ari[:], asl)
                    if abl_noar:
                        nc.sync.dma_start(aro[:], ari[:])
                    else:
                        nc.gpsimd.collective_compute(
                            "AllReduce", OP.add, replica_groups=[list(range(8))],
                            ins=[ari.opt()], outs=[aro.opt()])
                    nc.sync.dma_start(hsum[:, fc * B:(fc + 1) * B], aro[:])
            else:
                for fc in range(4):
                    asl = acc1[:, fc * B:(fc + 1) * B]
                    nc.vector.scalar_tensor_tensor(out=asl, in0=plate[:], scalar=1.0,
                                                   in1=asl, op0=OP.mult, op1=OP.add)

                # ---------------- allreduce layer 1 ----------------
                ar_in = dp.tile([128, 4 * B], F32, tag="ar_in%d" % _par)
                ar_out = dp.tile([128, 4 * B], F32, tag="ar_out%d" % _par)
                nc.sync.dma_start(ar_in[:], acc1[:])
                if abl_noar:
                    nc.sync.dma_start(ar_out[:], ar_in[:])
                else:
                    nc.gpsimd.collective_compute(
                        "AllReduce", OP.add, replica_groups=[list(range(8))],
                        ins=[ar_in.opt()], outs=[ar_out.opt()])
                nc.sync.dma_start(hsum[:], ar_out[:])

            # ---------------- tanh, hq, bits2 ----------------
            bits2 = cp.tile([128, 4 * B], BF16, tag="bits2")
            hqbf = cp.tile([128, 4 * B], BF16, tag="hqbf")
            for fc in range(4):
                ht = wp.tile([128, B], F32, tag="w32")
                nc.scalar.activation(ht[:], hsum[:, fc * B:(fc + 1) * B], ACTF.Tanh,
                                     bias=0.0, scale=KAPPA)
                if debug:
                    nc.sync.dma_start(hdbg[:, fc * B:(fc + 1) * B], ht[:])
                hc = wp.tile([128, B], F32, tag="w32")
                nc.vector.tensor_scalar(out=hc[:], in0=ht[:], scalar1=0.0, scalar2=1.0,
                                        op0=OP.max, op1=OP.min)
                hq = wp.tile([128, B], I32, tag="wi32")
                nc.vector.tensor_scalar(out=hq[:], in0=hc[:], scalar1=255.0, scalar2=0.0,
                                        op0=OP.mult, op1=OP.add)
                nc.vector.tensor_scalar(out=hqbf[:, fc * B:(fc + 1) * B], in0=hq[:],
                                        scalar1=1.0, scalar2=0.0, op0=OP.mult, op1=OP.add)
                bsh = wp.tile([128, B], I32, tag="wi32")
                nc.vector.tensor_scalar(out=bsh[:], in0=hq[:], scalar1=sb_cc[:, 5:6],
                                        scalar2=-0.499, op0=OP.mult, op1=OP.add)
                half = wp.tile([128, B], I32, tag="wi32")
                nc.vector.tensor_scalar(out=half[:], in0=bsh[:], scalar1=0.5,
                                        scalar2=-0.499, op0=OP.mult, op1=OP.add)
                nc.vector.scalar_tensor_tensor(out=bits2[:, fc * B:(fc + 1) * B],
                                               in0=half[:], scalar=-2.0, in1=bsh[:],
                                               op0=OP.mult, op1=OP.add)

            # hqsum row
            pshq = psM.tile([1, B], F32, tag="m")
            for fc in range(4):
                for h in range(BH):
                    nc.tensor.matmul(pshq[:, h * NB:(h + 1) * NB], sb_o128[:],
                                     hqbf[:, fc * B + h * NB: fc * B + (h + 1) * NB],
                                     start=(fc == 0), stop=(fc == 3))
            hrow = rp.tile([1, B], F32, tag="rowB")
            nc.vector.tensor_scalar(out=hrow[:], in0=pshq[:],
                                    scalar1=float(np.float32(-1.0 / (255.0 * KAPPA * 8.0))),
                                    scalar2=0.0, op0=OP.mult, op1=OP.add)

            # ---------------- dummy D2 ----------------
            psD2 = psM.tile([S2, B], F32, tag="m")
            for s in range(S2):
                for h in range(BH):
                    nc.tensor.matmul(psD2[:, h * NB:(h + 1) * NB],
                                     sb_e42[:, s * S2:(s + 1) * S2],
                                     bits2[:, s * B + h * NB: s * B + (h + 1) * NB],
                                     start=(s == 0), stop=(s == S2 - 1))
            D2sb = wp.tile([S2, B], F32, tag="w32")
            nc.vector.tensor_scalar(out=D2sb[:], in0=psD2[:], scalar1=C03, scalar2=0.0,
                                    op0=OP.mult, op1=OP.add)
            mxD2 = tp.tile([S2, 1], F32, tag="t2")
            mnD2 = tp.tile([S2, 1], F32, tag="t2")
            nc.vector.tensor_reduce(out=mxD2[:], in_=D2sb[:], axis=AX.X, op=OP.max)
            nc.vector.tensor_reduce(out=mnD2[:], in_=D2sb[:], axis=AX.X, op=OP.min)
            dD2 = tp.tile([S2, 1], F32, tag="t2")
            nc.vector.tensor_tensor(out=dD2[:], in0=mxD2[:], in1=mnD2[:], op=OP.subtract)
            rcD2 = tp.tile([S2, 1], F32, tag="t2")
            nc.vector.reciprocal(rcD2[:], dD2[:])
            mkD2 = tp.tile([S2, 1], F32, tag="t2")
            nc.vector.tensor_scalar(out=mkD2[:], in0=dD2[:], scalar1=0.0, scalar2=0.0,
                                    op0=OP.is_gt, op1=OP.add)
            rD2 = tp.tile([S2, 1], F32, tag="t2")
            nc.vector.tensor_scalar(out=rD2[:], in0=rcD2[:], scalar1=mkD2[:, 0:1],
                                    scalar2=RSCALE, op0=OP.mult, op1=OP.mult)
            rD2n = tp.tile([S2, 1], F32, tag="t2")
            nc.vector.tensor_scalar(out=rD2n[:], in0=rD2[:], scalar1=-1.0, scalar2=0.0,
                                    op0=OP.mult, op1=OP.add)
            bD2 = tp.tile([S2, 1], F32, tag="t2")
            nc.vector.tensor_scalar(out=bD2[:], in0=mnD2[:], scalar1=rD2n[:, 0:1],
                                    scalar2=-0.5, op0=OP.mult, op1=OP.add)
            stD2 = tp.tile([S2, 1], F32, tag="t2")
            nc.vector.tensor_scalar(out=stD2[:], in0=dD2[:], scalar1=STEPS, scalar2=0.0,
                                    op0=OP.mult, op1=OP.add)
            idxD2 = wp.tile([S2, B], I32, tag="wi32")
            nc.vector.tensor_scalar(out=idxD2[:], in0=D2sb[:], scalar1=rD2[:, 0:1],
                                    scalar2=bD2[:, 0:1], op0=OP.mult, op1=OP.add)
            DqD2 = wp.tile([S2, B], F32, tag="w32")
            nc.vector.tensor_scalar(out=DqD2[:], in0=idxD2[:], scalar1=stD2[:, 0:1],
                                    scalar2=mnD2[:, 0:1], op0=OP.mult, op1=OP.add)

            # ---------------- layer-2 main ----------------
            def emit_p2(ps, s2):
                wof = s2 * KSL * F2
                for h in range(BH):
                    nc.tensor.matmul(ps[:, h * NB:(h + 1) * NB],
                                     sb_g2h[:, wof:wof + KSL * F2],
                                     bits2[:, s2 * B + h * NB: s2 * B + (h + 1) * NB],
                                     start=True, stop=False)
                for h in range(BH):
                    nc.tensor.matmul(ps[:, h * NB:(h + 1) * NB],
                                     sb_g2l[:, wof:wof + KSL * F2],
                                     bits2[:, s2 * B + h * NB: s2 * B + (h + 1) * NB],
                                     start=False, stop=True)

            M2 = KSL * F2
            maxC2 = lp.tile([M2, 2 * S2], F32, tag="c2")
            for s2 in range(S2):
                ps = psA.tile([M2, B], F32, tag="p1")
                emit_p2(ps, s2)
                nc.vector.tensor_reduce(out=maxC2[:, s2:s2 + 1], in_=ps[:],
                                        axis=AX.X, op=OP.max)
                nc.vector.tensor_reduce(out=maxC2[:, S2 + s2:S2 + s2 + 1], in_=ps[:],
                                        axis=AX.X, op=OP.min)
            pt2a = psM.tile([S2, M2], F32, tag="m")
            nc.tensor.transpose(pt2a[:], maxC2[:, 0:S2], sb_id[0:M2, 0:M2])
            tca = tp.tile([S2, M2], F32, tag="tcc")
            nc.vector.tensor_copy(tca[:], pt2a[:])
            pt2b = psM.tile([S2, M2], F32, tag="m")
            nc.tensor.transpose(pt2b[:], maxC2[:, S2:2 * S2], sb_id[0:M2, 0:M2])
            tcb = tp.tile([S2, M2], F32, tag="tcc")
            nc.vector.tensor_copy(tcb[:], pt2b[:])
            mx2 = tp.tile([S2, KSL], F32, tag="q")
            mn2 = tp.tile([S2, KSL], F32, tag="q")
            nc.vector.tensor_reduce(out=mx2[:],
                                    in_=tca[:].rearrange("p (k f) -> p k f", f=F2),
                                    axis=AX.X, op=OP.max)
            nc.vector.tensor_reduce(out=mn2[:],
                                    in_=tcb[:].rearrange("p (k f) -> p k f", f=F2),
                                    axis=AX.X, op=OP.min)
            d2 = tp.tile([S2, KSL], F32, tag="q")
            nc.vector.tensor_tensor(out=d2[:], in0=mx2[:], in1=mn2[:], op=OP.subtract)
            rc2 = tp.tile([S2, KSL], F32, tag="q")
            nc.vector.reciprocal(rc2[:], d2[:])
            mk2 = tp.tile([S2, KSL], F32, tag="q")
            nc.vector.tensor_scalar(out=mk2[:], in0=d2[:], scalar1=0.0, scalar2=0.0,
                                    op0=OP.is_gt, op1=OP.add)
            r2t = tp.tile([S2, KSL], F32, tag="q")
            nc.vector.tensor_tensor(out=r2t[:], in0=rc2[:], in1=mk2[:], op=OP.mult)
            nc.vector.tensor_scalar(out=r2t[:], in0=r2t[:], scalar1=RSCALE, scalar2=0.0,
                                    op0=OP.mult, op1=OP.add)
            b2t = tp.tile([S2, KSL], F32, tag="q")
            nc.vector.tensor_tensor(out=b2t[:], in0=mn2[:], in1=r2t[:], op=OP.mult)
            nc.vector.tensor_scalar(out=b2t[:], in0=b2t[:], scalar1=-1.0, scalar2=-0.5,
                                    op0=OP.mult, op1=OP.add)
            st2t = tp.tile([S2, KSL], F32, tag="q")
            nc.vector.tensor_scalar(out=st2t[:], in0=d2[:], scalar1=STEPS, scalar2=0.0,
                                    op0=OP.mult, op1=OP.add)
            c2t = tp.tile([S2, KSL], F32, tag="q")
            nc.vector.tensor_tensor(out=c2t[:], in0=st2t[:], in1=sb_s2t[:], op=OP.mult)
            smn2 = tp.tile([S2, KSL], F32, tag="q")
            nc.vector.tensor_tensor(out=smn2[:], in0=mn2[:], in1=sb_s2t[:], op=OP.mult)
            psg2 = psM.tile([1, KSL], F32, tag="m")
            nc.tensor.matmul(psg2[:], sb_o7[0:S2, :], smn2[:], start=True, stop=True)
            sg2r = tp.tile([1, KSL], F32, tag="sg")
            nc.vector.tensor_copy(sg2r[:], psg2[:])
            sg2 = tp.tile([1, 1], F32, tag="sg1")
            nc.vector.tensor_reduce(out=sg2[:], in_=sg2r[:], axis=AX.X, op=OP.add)
            # transpose r2/b2/c2 ([s2,k] -> [k,s2]) via PE, then per-s2 bcast
            psT = psM.tile([S2, 3 * KSL], F32, tag="m")
            nc.tensor.transpose(psT[:, 0:KSL], r2t[:], sb_id[0:S2, 0:S2])
            nc.tensor.transpose(psT[:, KSL:2 * KSL], b2t[:], sb_id[0:S2, 0:S2])
            nc.tensor.transpose(psT[:, 2 * KSL:3 * KSL], c2t[:], sb_id[0:S2, 0:S2])
            sT = tp.tile([S2, 3 * KSL], F32, tag="sT")
            nc.vector.tensor_copy(sT[:], psT[:])
            scl2 = []
            for s2 in range(S2):
                rhs = tp.tile([KSL, 3], F32, tag="rh")
                nc.vector.tensor_copy(rhs[:, 0:1], sT[:, s2:s2 + 1])
                nc.vector.tensor_copy(rhs[:, 1:2], sT[:, KSL + s2:KSL + s2 + 1])
                nc.vector.tensor_copy(rhs[:, 2:3], sT[:, 2 * KSL + s2:2 * KSL + s2 + 1])
                psc = psM.tile([M2, 3], F32, tag="m")
                nc.tensor.matmul(psc[:], sb_bind[:], rhs[:], start=True, stop=True)
                sc = tp.tile([M2, 3], F32, tag="sc%d" % s2)
                nc.vector.tensor_copy(sc[:], psc[:])
                scl2.append(sc)
            acc2 = cp.tile([M2, B], F32, tag="acc2")
            for s2 in range(S2):
                ps = psB.tile([M2, B], F32, tag="p2")
                emit_p2(ps, s2)
                idxt = ip.tile([M2, B], I32, tag="ix")
                nc.scalar.activation(idxt[:], ps[:], ACTF.Identity,
                                     bias=scl2[s2][:, 1:2], scale=scl2[s2][:, 0:1])
                if s2 == 0:
                    nc.vector.tensor_scalar(out=acc2[:], in0=idxt[:],
                                            scalar1=scl2[s2][:, 2:3], scalar2=0.0,
                                            op0=OP.mult, op1=OP.add)
                else:
                    nc.vector.scalar_tensor_tensor(out=acc2[:], in0=idxt[:],
                                                   scalar=scl2[s2][:, 2:3], in1=acc2[:],
                                                   op0=OP.mult, op1=OP.add)
            # dummy2 row + sigma2 + hq row -> late2row
            psd2 = psM.tile([1, B], F32, tag="m")
            for h in range(BH):
                nc.tensor.matmul(psd2[:, h * NB:(h + 1) * NB], sb_o7[0:S2, :],
                                 DqD2[:, h * NB:(h + 1) * NB], start=True, stop=True)
            l2a = rp.tile([1, B], F32, tag="rowB")
            nc.vector.tensor_scalar(out=l2a[:], in0=psd2[:], scalar1=sb_cc[0:1, 4:5],
                                    scalar2=sg2[:, 0:1], op0=OP.mult, op1=OP.add)
            l2b = rp.tile([1, B], F32, tag="rowB")
            nc.vector.tensor_tensor(out=l2b[:], in0=l2a[:], in1=hrow[:], op=OP.add)
            # fold 40 -> 10 plus rank-1 late row
            psf = psM.tile([F2, B], F32, tag="m")
            for h in range(BH):
                nc.tensor.matmul(psf[:, h * NB:(h + 1) * NB], sb_f40[:],
                                 acc2[:, h * NB:(h + 1) * NB], start=True, stop=False)
            for h in range(BH):
                nc.tensor.matmul(psf[:, h * NB:(h + 1) * NB], sb_orow[0:1, 0:F2],
                                 l2b[:, h * NB:(h + 1) * NB], start=False, stop=True)
            o2a = rp.tile([F2, B], F32, tag="rowB")
            nc.vector.tensor_copy(o2a[:], psf[:])
            ar2i = dp.tile([F2, B], F32, tag="ar2i%d" % _par)
            ar2o = dp.tile([F2, B], F32, tag="ar2o%d" % _par)
            nc.sync.dma_start(ar2i[:], o2a[:])
            if abl_noar:
                nc.sync.dma_start(ar2o[:], ar2i[:])
            else:
                nc.gpsimd.collective_compute(
                    "AllReduce", OP.add, replica_groups=[list(range(8))],
                    ins=[ar2i.opt()], outs=[ar2o.opt()])
            o2b = rp.tile([F2, B], F32, tag="rowB")
            nc.sync.dma_start(o2b[:], ar2o[:])
            o2c = rp.tile([F2, B], F32, tag="rowB")
            nc.vector.tensor_scalar(out=o2c[:], in0=o2b[:], scalar1=KAPPA, scalar2=0.0,
                                    op0=OP.mult, op1=OP.add)
            nc.sync.dma_start(out2d[:], o2c[:])

    nc.compile()
    _NC_CACHE[key] = nc
    return nc


# ------------------------------------------------------------------ runner
class Runner:
    """Cached jit over the bass_exec custom call with device-resident inputs.

    Same lowering path bass_utils.run_bass_kernel_spmd takes under axon
    (bass2jax.run_bass_via_pjrt), but the jitted callable and the
    device-placed input buffers persist across calls, so a warm call ships
    no input data over the axon tunnel.
    """

    def __init__(self, nc):
        install_neuronx_cc_hook()
        self.nc = nc
        partition_name = (nc.partition_id_tensor.name
                          if nc.partition_id_tensor else None)
        in_names, out_names, out_avals, zero_shapes = [], [], [], []
        for alloc in nc.m.functions[0].allocations:
            if not isinstance(alloc, mybir.MemoryLocationSet):
                continue
            name = alloc.memorylocations[0].name
            if alloc.kind == "ExternalInput":
                if name != partition_name:
                    in_names.append(name)
            elif alloc.kind == "ExternalOutput":
                out_names.append(name)
                shape = tuple(alloc.tensor_shape)
                dtype = mybir.dt.np(alloc.dtype)
                out_avals.append(jax.core.ShapedArray(shape, dtype))
                zero_shapes.append((shape, dtype))
        self.in_names = in_names
        self.out_names = out_names
        n_params = len(in_names)
        all_names = list(in_names) + list(out_names)
        if partition_name is not None:
            all_names.append(partition_name)

        def _body(*args):
            operands = list(args)
            if partition_name is not None:
                operands.append(partition_id_tensor())
            return tuple(_bass_exec_p.bind(
                *operands,
                out_avals=tuple(out_avals),
                in_names=tuple(all_names),
                out_names=tuple(out_names),
                lowering_input_output_aliases=(),
                sim_require_finite=True,
                sim_require_nnan=True,
                nc=nc,
            ))

        devices = jax.devices()[:N_CORES]
        self.mesh = Mesh(np.asarray(devices), ("core",))
        n_outs = len(out_names)
        in_specs = (PartitionSpec("core"),) * (n_params + n_outs)
        out_specs = (PartitionSpec("core"),) * n_outs
        self.fn = jax.jit(shard_map(_body, mesh=self.mesh, in_specs=in_specs,
                                    out_specs=out_specs, check_rep=False))
        self.sharding = NamedSharding(self.mesh, PartitionSpec("core"))
        self.dev_zeros = [
            jax.device_put(np.zeros((N_CORES * s[0], *s[1:]), dt), self.sharding)
            for s, dt in zero_shapes]
        self.dev_in = None

    def put_inputs(self, in_maps):
        """Concat per-core input maps and place on devices (sharded by core)."""
        self.dev_in = [
            jax.device_put(
                np.concatenate([np.asarray(m[name]) for m in in_maps], axis=0),
                self.sharding)
            for name in self.in_names]
        jax.block_until_ready(self.dev_in)

    def execute(self):
        """One dispatch (async); returns jax output arrays, concat by core."""
        return self.fn(*self.dev_in, *self.dev_zeros)

    def run(self):
        outs = self.execute()
        jax.block_until_ready(outs)
        return {name: np.asarray(outs[i]) for i, name in enumerate(self.out_names)}


def get_runner(B, nrep=1, debug=False, store_p=0, gp_accum=False,
               abl_nop2mm=False, abl_noar=False, gp_minred=False, gp_max=False,
             ar_chunk=False, mm1024=False):
    key = (B, nrep, debug, store_p, gp_accum, abl_nop2mm, abl_noar, gp_minred,
           gp_max, ar_chunk, mm1024)
    if key not in _RUNNER_CACHE:
        _RUNNER_CACHE[key] = Runner(build_nc(
            B, nrep=nrep, debug=debug, store_p=store_p, gp_accum=gp_accum,
            abl_nop2mm=abl_nop2mm, abl_noar=abl_noar, gp_minred=gp_minred))
    return _RUNNER_CACHE[key]


# ------------------------------------------------------------------ driver
def run_cores(inputs, B=1024, want_debug=False, nrep=1):
    shared, per_core = host_prepare(inputs["x"], inputs["w1"], inputs["w3"],
                                    inputs["noise1"], inputs["noise3"], B)
    runner = get_runner(B, nrep=nrep, debug=want_debug, gp_max=True)
    runner.put_inputs([{**shared, **pc} for pc in per_core])
    res = runner.run()
    out = res["out2"][0:F2].T.astype(np.float32)  # core 0 slice, [B, F2]
    if want_debug:
        return out, res
    return out


def kernel(**inputs):
    return run_cores(inputs, B=1024)
